# revision 1
# baseline (speedup 1.0000x reference)
import numpy as np

N_NODES = 50000
N_GRAPHS = 128
N_CONV = 2
NEG_SLOPE = 0.01
HIDDEN = 32


def _kan(x, W, bias=None):
    # x: (B, in), W: (2, out, in, grid) -> (B, out)
    x = np.asarray(x, dtype=np.float32)
    W = np.asarray(W, dtype=np.float32)
    g = W.shape[-1]
    k = np.arange(1, g + 1, dtype=np.float32)
    arg = x[:, :, None] * k                      # (B, in, g)
    B = x.shape[0]
    co = np.cos(arg).reshape(B, -1)              # (B, in*g), (i,g) order
    si = np.sin(arg).reshape(B, -1)
    out_dim = W.shape[1]
    W0 = W[0].reshape(out_dim, -1)               # (out, in*g), (i,g) order
    W1 = W[1].reshape(out_dim, -1)
    y = co @ W0.T + si @ W1.T
    if bias is not None:
        y = y + np.asarray(bias, dtype=np.float32)
    return y.astype(np.float32)


def _segment_sum(values, seg_ids, n_segments):
    # values: (E, F) float32, seg_ids: (E,) int -> (n_segments, F)
    F = values.shape[1]
    out = np.empty((n_segments, F), dtype=np.float32)
    for f in range(F):
        out[:, f] = np.bincount(
            seg_ids, weights=values[:, f].astype(np.float64), minlength=n_segments
        )[:n_segments]
    return out


def kernel(x, edge_index, batch, W_in, W_conv, W_out, b_out):
    x = np.asarray(x, dtype=np.float32)
    edge_index = np.asarray(edge_index)
    batch = np.asarray(batch)
    W_in = np.asarray(W_in, dtype=np.float32)
    W_conv = np.asarray(W_conv, dtype=np.float32)
    W_out = np.asarray(W_out, dtype=np.float32)
    b_out = np.asarray(b_out, dtype=np.float32)

    src = edge_index[0].astype(np.int64)
    dst = edge_index[1].astype(np.int64)
    bat = batch.astype(np.int64)
    n_nodes = x.shape[0]

    # input KAN projection
    h = _kan(x, W_in)

    # message-passing layers: kan_apply(h[src]) == kan_apply(h)[src] since the
    # transform is row-wise -> compute per node (50k rows) instead of per edge (800k)
    for l in range(N_CONV):
        msg = _kan(h, W_conv[l])
        m = _segment_sum(msg[src], dst, n_nodes)
        z = m + h
        h = np.where(z >= 0, z, NEG_SLOPE * z).astype(np.float32)

    # mean pool per graph
    n_graphs = N_GRAPHS
    sums = _segment_sum(h, bat, n_graphs)
    counts = np.bincount(bat, minlength=n_graphs)[:n_graphs].astype(np.float32)
    y = sums / np.maximum(counts, 1.0)[:, None]

    # readout KAN_linear (grid=1, bias) + sigmoid
    out = _kan(y, W_out, b_out)
    out = 1.0 / (1.0 + np.exp(-out.astype(np.float32)))
    return out.astype(np.float32)



# revision 2
# speedup vs baseline: 3.2381x; 3.2381x over previous
"""KA-GNN (Fourier-KAN message passing) on 8 Trainium2 NeuronCores via Bass.

Strategy (sharding_hint: partition nodes/edges across cores, replicate small
Fourier coefficients, collective only at readout):
 - nodes are split into 128-row tiles; each core owns T=49 consecutive tiles
   (dst buckets) -> node/edge partition across 8 cores.
 - per conv layer each core computes Fourier-KAN messages for its own nodes
   (Chebyshev recurrence on-device: cos/sin(k*h), k=1..4), writes a fp16 msg
   table slice to DRAM, AllGather -> full table.
 - scatter-add aggregation: edges bucketed by dst tile (host-side O(E) radix
   grouping), per 128-edge chunk one indirect-DMA row gather + one-hot
   selection matrix (is_equal vs iota) matmul accumulated in PSUM.
 - mean-pool partial sums per core -> AllReduce -> replicated KAN readout.

Import-time: trace + neuronx-cc compile (disk-cached) + warm-up execution.
kernel() call: host preprocessing + one sharded device invocation.
Any failure falls back to a pure-NumPy implementation.
"""

import numpy as np

P = 128
N_CORES = 8
T = 49                 # node tiles (dst buckets) per core
CH = 18                # max 128-edge chunks per bucket
INF = 64
HID = 32
GRID = 4
N_CONV = 2
NCH = 6                # trig free-dim chunking
N_NODES = 50000
N_GRAPHS = 128
NEG_SLOPE = 0.01
NPC = T * P
NPAD = NPC * N_CORES

MAGIC = 1.5 * 2.0**23
INV2PI = float(1.0 / (2.0 * np.pi))
TWOPI = float(2.0 * np.pi)

_STATE = {}


# ======================= device program =======================

def _build_nc():
    from contextlib import ExitStack
    import concourse.bass as bass
    import concourse.bacc as bacc
    import concourse.mybir as mybir
    import concourse.tile as tile
    from concourse.masks import make_identity

    F32 = mybir.dt.float32
    F16 = mybir.dt.float16
    I32 = mybir.dt.int32
    U16 = mybir.dt.uint16
    A = mybir.AluOpType
    ACT = mybir.ActivationFunctionType

    KIN = 2 * INF
    KCV = GRID * HID

    nc = bacc.Bacc("TRN2", target_bir_lowering=False, debug=False,
                   num_devices=N_CORES)

    xtb_d = nc.dram_tensor("xtb", [INF, NPC], F16, kind="ExternalInput").ap()
    srcp_d = nc.dram_tensor("srcp", [P, T * CH], U16, kind="ExternalInput").ap()
    dlob_d = nc.dram_tensor("dlob", [P, T * CH], F16, kind="ExternalInput").ap()
    batch_d = nc.dram_tensor("batchb", [P, T], F16, kind="ExternalInput").ap()
    win_d = nc.dram_tensor("win", [4, KIN, HID], F16, kind="ExternalInput").ap()
    wconv_d = nc.dram_tensor("wconv", [N_CONV, 2, KCV, HID], F16,
                             kind="ExternalInput").ap()
    wro_d = nc.dram_tensor("wro", [P, 2 * HID + 1], F32, kind="ExternalInput").ap()
    out_d = nc.dram_tensor("out", [P, 1], F32, kind="ExternalOutput").ap()

    replica = [list(range(N_CORES))]
    CHW = (T + NCH - 1) // NCH

    with tile.TileContext(nc) as tc, ExitStack() as ctx:
        sb = ctx.enter_context(tc.tile_pool(name="sb", bufs=1))
        tg = ctx.enter_context(tc.tile_pool(name="tg", bufs=1))
        sbg = ctx.enter_context(tc.tile_pool(name="sbg", bufs=4))
        sel_p = ctx.enter_context(tc.tile_pool(name="selp", bufs=2))
        pst = ctx.enter_context(tc.tile_pool(name="pst", bufs=2, space="PSUM"))
        psa = ctx.enter_context(tc.tile_pool(name="psa", bufs=2, space="PSUM"))
        dr = ctx.enter_context(tc.tile_pool(name="dr", bufs=1, space="DRAM"))

        # ---------------- setup ----------------
        xtb = sb.tile([INF, NPC], F16)
        nc.sync.dma_start(out=xtb[:], in_=xtb_d[:])
        srcp_u = sb.tile([P, T * CH], U16)
        nc.sync.dma_start(out=srcp_u[:], in_=srcp_d[:])
        srci = sb.tile([P, T * CH], I32)
        nc.vector.tensor_copy(srci[:], srcp_u[:])
        dlo = sb.tile([P, T * CH], F16)
        nc.sync.dma_start(out=dlo[:], in_=dlob_d[:])
        batchb = sb.tile([P, T], F16)
        nc.sync.dma_start(out=batchb[:], in_=batch_d[:])

        win_b = sb.tile([P, 4 * HID], F16)
        for b in range(4):
            nc.sync.dma_start(out=win_b[:KIN, b * HID:(b + 1) * HID], in_=win_d[b])
        wconv_b = sb.tile([P, N_CONV * 2 * HID], F16)
        for l in range(N_CONV):
            for s in range(2):
                c0 = (l * 2 + s) * HID
                nc.sync.dma_start(out=wconv_b[:KCV, c0:c0 + HID], in_=wconv_d[l, s])
        wro = sb.tile([P, 2 * HID + 1], F32)
        nc.sync.dma_start(out=wro[:], in_=wro_d[:])

        iota_i = sb.tile([P, P], I32)
        nc.gpsimd.iota(iota_i[:], pattern=[[1, P]], base=0, channel_multiplier=0)
        iota_h = sb.tile([P, P], F16)
        nc.vector.tensor_copy(iota_h[:], iota_i[:])
        ident = sb.tile([P, P], F32)
        make_identity(nc, ident[:])

        locl = [dr.tile([NPC, HID], F16, name=f"locl{i}") for i in range(N_CONV)]
        tbl = [dr.tile([NPAD, HID], F16, name=f"tbl{i}") for i in range(N_CONV)]
        arin = dr.tile([P, HID + 1], F32)
        arout = dr.tile([P, HID + 1], F32)

        h_buf_a = sb.tile([P, T * HID], F32, tag="hbuf_a")
        h_buf_b = sb.tile([P, T * HID], F32, tag="hbuf_b")
        TcA = sb.tile([P, NPC], F16, tag="TcA")
        TcB = sb.tile([P, NPC], F16, tag="TcB")
        TsA = sb.tile([P, NPC], F16, tag="TsA")
        TsB = sb.tile([P, NPC], F16, tag="TsB")

        def chebyshev(src, rows, cols, col0, dst_of):
            u = tg.tile([rows, CHW * P], F32, tag="ch_u")
            r = tg.tile([rows, CHW * P], F32, tag="ch_r")
            nc.vector.tensor_scalar(out=r[:rows, :cols], in0=src, scalar1=INV2PI,
                                    scalar2=MAGIC, op0=A.mult, op1=A.add)
            nc.vector.tensor_scalar(out=r[:rows, :cols], in0=r[:rows, :cols],
                                    scalar1=MAGIC, op0=A.subtract, scalar2=None)
            nc.vector.tensor_scalar(out=r[:rows, :cols], in0=r[:rows, :cols],
                                    scalar1=-TWOPI, op0=A.mult, scalar2=None)
            nc.vector.tensor_tensor(out=u[:rows, :cols], in0=src,
                                    in1=r[:rows, :cols], op=A.add)
            s1 = tg.tile([rows, CHW * P], F32, tag="ch_s1")
            chh = tg.tile([rows, CHW * P], F32, tag="ch_ch")
            c1 = tg.tile([rows, CHW * P], F32, tag="ch_c1")
            nc.scalar.activation(s1[:rows, :cols], u[:rows, :cols], ACT.Sin)
            nc.scalar.activation(chh[:rows, :cols], u[:rows, :cols], ACT.Sin,
                                 scale=0.5)
            nc.vector.tensor_tensor(out=c1[:rows, :cols], in0=chh[:rows, :cols],
                                    in1=chh[:rows, :cols], op=A.mult)
            nc.vector.tensor_scalar(out=c1[:rows, :cols], in0=c1[:rows, :cols],
                                    scalar1=-2.0, scalar2=1.0, op0=A.mult,
                                    op1=A.add)
            twoc = tg.tile([rows, CHW * P], F32, tag="ch_twoc")
            nc.vector.tensor_scalar(out=twoc[:rows, :cols], in0=c1[:rows, :cols],
                                    scalar1=2.0, op0=A.mult, scalar2=None)
            cs = {1: (c1, s1)}
            prev_c, prev_s = c1, s1
            pprev_c, pprev_s = None, None
            for k in range(2, GRID + 1):
                ck = tg.tile([rows, CHW * P], F32, tag=f"ch_c{k}")
                sk = tg.tile([rows, CHW * P], F32, tag=f"ch_s{k}")
                nc.vector.tensor_tensor(out=ck[:rows, :cols],
                                        in0=twoc[:rows, :cols],
                                        in1=prev_c[:rows, :cols], op=A.mult)
                nc.vector.tensor_tensor(out=sk[:rows, :cols],
                                        in0=twoc[:rows, :cols],
                                        in1=prev_s[:rows, :cols], op=A.mult)
                if k == 2:
                    nc.vector.tensor_scalar(out=ck[:rows, :cols],
                                            in0=ck[:rows, :cols], scalar1=1.0,
                                            op0=A.subtract, scalar2=None)
                else:
                    nc.vector.tensor_tensor(out=ck[:rows, :cols],
                                            in0=ck[:rows, :cols],
                                            in1=pprev_c[:rows, :cols],
                                            op=A.subtract)
                    nc.vector.tensor_tensor(out=sk[:rows, :cols],
                                            in0=sk[:rows, :cols],
                                            in1=pprev_s[:rows, :cols],
                                            op=A.subtract)
                cs[k] = (ck, sk)
                pprev_c, pprev_s = prev_c, prev_s
                prev_c, prev_s = ck, sk
            for k in range(1, GRID + 1):
                (ctile, r0), (stile, r1) = dst_of(k)
                ckf, skf = cs[k]
                nc.vector.tensor_copy(ctile[r0:r0 + rows, col0:col0 + cols],
                                      ckf[:rows, :cols])
                nc.vector.tensor_copy(stile[r1:r1 + rows, col0:col0 + cols],
                                      skf[:rows, :cols])

        # ---------- input layer ----------
        def in_dst(k):
            if k <= 2:
                return (TcA, (k - 1) * INF), (TsA, (k - 1) * INF)
            return (TcB, (k - 3) * INF), (TsB, (k - 3) * INF)

        for c0 in range(0, T, CHW):
            c1_ = min(c0 + CHW, T)
            cols = (c1_ - c0) * P
            chebyshev(xtb[:INF, c0 * P:c0 * P + cols], INF, cols, c0 * P, in_dst)

        h_cur, h_nxt = h_buf_a, h_buf_b
        for t in range(T):
            ph = psa.tile([P, HID], F32, tag="ps_acc")
            blocks = [(TcA, 0), (TcB, 1), (TsA, 2), (TsB, 3)]
            for i, (tt, b) in enumerate(blocks):
                nc.tensor.matmul(out=ph[:], lhsT=tt[:KIN, t * P:(t + 1) * P],
                                 rhs=win_b[:KIN, b * HID:(b + 1) * HID],
                                 start=(i == 0), stop=(i == 3))
            nc.vector.tensor_copy(h_cur[:, t * HID:(t + 1) * HID], ph[:])

        # ---------- conv layers ----------
        def conv_dst(k):
            return (TcA, (k - 1) * HID), (TcB, (k - 1) * HID)

        for l in range(N_CONV):
            for c0 in range(0, T, CHW):
                c1_ = min(c0 + CHW, T)
                cols = (c1_ - c0) * P
                h_T = tg.tile([HID, CHW * P], F32, tag="h_T")
                for t in range(c0, c1_):
                    pt = pst.tile([HID, P], F32, tag="ps_tr")
                    nc.tensor.transpose(out=pt[:],
                                        in_=h_cur[:, t * HID:(t + 1) * HID],
                                        identity=ident[:])
                    nc.vector.tensor_copy(
                        h_T[:, (t - c0) * P:(t - c0 + 1) * P], pt[:])
                chebyshev(h_T[:HID, :cols], HID, cols, c0 * P, conv_dst)

            msg_sb = sb.tile([P, T * HID], F16, tag="msg_sb")
            for t in range(T):
                pm = psa.tile([P, HID], F32, tag="ps_acc")
                nc.tensor.matmul(out=pm[:], lhsT=TcA[:KCV, t * P:(t + 1) * P],
                                 rhs=wconv_b[:KCV, (l * 2) * HID:(l * 2 + 1) * HID],
                                 start=True, stop=False)
                nc.tensor.matmul(out=pm[:], lhsT=TcB[:KCV, t * P:(t + 1) * P],
                                 rhs=wconv_b[:KCV, (l * 2 + 1) * HID:(l * 2 + 2) * HID],
                                 start=False, stop=True)
                nc.vector.tensor_copy(msg_sb[:, t * HID:(t + 1) * HID], pm[:])
            nc.gpsimd.dma_start(
                out=locl[l][:].rearrange("(t p) f -> p t f", p=P),
                in_=msg_sb[:].rearrange("p (t f) -> p t f", t=T))
            nc.gpsimd.collective_compute(
                "AllGather", A.bypass, replica_groups=replica,
                ins=[locl[l].opt()], outs=[tbl[l].opt()])

            for b in range(T):
                sel = sel_p.tile([P, CH * P], F16, tag="sel")
                nc.vector.tensor_tensor(
                    out=sel[:].rearrange("p (c q) -> p c q", c=CH),
                    in0=dlo[:, b * CH:(b + 1) * CH]
                        .rearrange("p (c o) -> p c o", o=1)
                        .to_broadcast([P, CH, P]),
                    in1=iota_h[:].rearrange("p (o q) -> p o q", o=1)
                        .to_broadcast([P, CH, P]),
                    op=A.is_equal)
                pagg = psa.tile([P, HID], F32, tag="ps_agg")
                for j in range(CH):
                    g_t = sbg.tile([P, HID], F16, tag="gath")
                    nc.gpsimd.indirect_dma_start(
                        out=g_t[:], out_offset=None, in_=tbl[l][:],
                        in_offset=bass.IndirectOffsetOnAxis(
                            ap=srci[:, b * CH + j:b * CH + j + 1], axis=0))
                    nc.tensor.matmul(out=pagg[:],
                                     lhsT=sel[:, j * P:(j + 1) * P],
                                     rhs=g_t[:], start=(j == 0),
                                     stop=(j == CH - 1))
                z = sbg.tile([P, HID], F32, tag="zt")
                nc.vector.tensor_tensor(out=z[:], in0=pagg[:],
                                        in1=h_cur[:, b * HID:(b + 1) * HID],
                                        op=A.add)
                zs = sbg.tile([P, HID], F32, tag="zs")
                nc.vector.tensor_scalar(out=zs[:], in0=z[:], scalar1=NEG_SLOPE,
                                        op0=A.mult, scalar2=None)
                nc.vector.tensor_tensor(out=h_nxt[:, b * HID:(b + 1) * HID],
                                        in0=z[:], in1=zs[:], op=A.max)
            h_cur, h_nxt = h_nxt, h_cur

        # ---------- pooling ----------
        h2b = sb.tile([P, T * (HID + 1)], F16, tag="h2b")
        for t in range(T):
            nc.vector.tensor_copy(
                h2b[:, t * (HID + 1):t * (HID + 1) + HID],
                h_cur[:, t * HID:(t + 1) * HID])
            nc.gpsimd.memset(h2b[:, t * (HID + 1) + HID:(t + 1) * (HID + 1)], 1.0)
        ppool = psa.tile([P, HID + 1], F32, tag="ps_acc")
        for t in range(T):
            selB = sbg.tile([P, P], F16, tag="selB")
            nc.vector.tensor_tensor(
                out=selB[:],
                in0=batchb[:, t:t + 1].to_broadcast([P, P]),
                in1=iota_h[:], op=A.is_equal)
            nc.tensor.matmul(out=ppool[:], lhsT=selB[:],
                             rhs=h2b[:, t * (HID + 1):(t + 1) * (HID + 1)],
                             start=(t == 0), stop=(t == T - 1))
        pool_sb = sb.tile([P, HID + 1], F32)
        nc.vector.tensor_copy(pool_sb[:], ppool[:])
        nc.gpsimd.dma_start(out=arin[:], in_=pool_sb[:])
        nc.gpsimd.collective_compute(
            "AllReduce", A.add, replica_groups=replica,
            ins=[arin.opt()], outs=[arout.opt()])
        sums = sb.tile([P, HID + 1], F32)
        nc.gpsimd.dma_start(out=sums[:], in_=arout[:])

        cnt = sb.tile([P, 1], F32)
        nc.vector.tensor_scalar(out=cnt[:], in0=sums[:, HID:HID + 1],
                                scalar1=1.0, op0=A.max, scalar2=None)
        rec = sb.tile([P, 1], F32)
        nc.vector.reciprocal(rec[:], cnt[:])
        y = sb.tile([P, HID], F32)
        nc.vector.tensor_tensor(out=y[:], in0=sums[:, :HID],
                                in1=rec[:].to_broadcast([P, HID]), op=A.mult)

        ur = sb.tile([P, HID], F32)
        rr_ = sb.tile([P, HID], F32)
        nc.vector.tensor_scalar(out=rr_[:], in0=y[:], scalar1=INV2PI,
                                scalar2=MAGIC, op0=A.mult, op1=A.add)
        nc.vector.tensor_scalar(out=rr_[:], in0=rr_[:], scalar1=MAGIC,
                                op0=A.subtract, scalar2=None)
        nc.vector.tensor_scalar(out=rr_[:], in0=rr_[:], scalar1=-TWOPI,
                                op0=A.mult, scalar2=None)
        nc.vector.tensor_tensor(out=ur[:], in0=y[:], in1=rr_[:], op=A.add)
        sy = sb.tile([P, HID], F32)
        chy = sb.tile([P, HID], F32)
        cy = sb.tile([P, HID], F32)
        nc.scalar.activation(sy[:], ur[:], ACT.Sin)
        nc.scalar.activation(chy[:], ur[:], ACT.Sin, scale=0.5)
        nc.vector.tensor_tensor(out=cy[:], in0=chy[:], in1=chy[:], op=A.mult)
        nc.vector.tensor_scalar(out=cy[:], in0=cy[:], scalar1=-2.0, scalar2=1.0,
                                op0=A.mult, op1=A.add)
        dotc = sb.tile([P, HID], F32)
        accc = sb.tile([P, 1], F32)
        nc.vector.tensor_tensor(out=dotc[:], in0=cy[:], in1=wro[:, 0:HID],
                                op=A.mult)
        nc.vector.tensor_reduce(out=accc[:], in_=dotc[:],
                                axis=mybir.AxisListType.X, op=A.add)
        dots = sb.tile([P, HID], F32)
        accs = sb.tile([P, 1], F32)
        nc.vector.tensor_tensor(out=dots[:], in0=sy[:], in1=wro[:, HID:2 * HID],
                                op=A.mult)
        nc.vector.tensor_reduce(out=accs[:], in_=dots[:],
                                axis=mybir.AxisListType.X, op=A.add)
        o_t = sb.tile([P, 1], F32)
        nc.vector.tensor_tensor(out=o_t[:], in0=accc[:], in1=accs[:], op=A.add)
        nc.vector.tensor_tensor(out=o_t[:], in0=o_t[:],
                                in1=wro[:, 2 * HID:2 * HID + 1], op=A.add)
        res = sb.tile([P, 1], F32)
        nc.scalar.activation(res[:], o_t[:], ACT.Sigmoid)
        nc.sync.dma_start(out=out_d[:], in_=res[:])

    nc.compile()
    return nc


def _make_runner(nc):
    import jax
    from jax.sharding import Mesh, PartitionSpec
    from jax.experimental.shard_map import shard_map
    import concourse.mybir as mybir
    from concourse import bass2jax

    bass2jax.install_neuronx_cc_hook()
    pname = nc.partition_id_tensor.name if nc.partition_id_tensor else None
    in_names, out_names, out_avals, zero_shapes = [], [], [], []
    for alloc in nc.m.functions[0].allocations:
        if not isinstance(alloc, mybir.MemoryLocationSet):
            continue
        name = alloc.memorylocations[0].name
        if alloc.kind == "ExternalInput":
            if name != pname:
                in_names.append(name)
        elif alloc.kind == "ExternalOutput":
            out_names.append(name)
            shape = tuple(alloc.tensor_shape)
            dtype = mybir.dt.np(alloc.dtype)
            out_avals.append(jax.core.ShapedArray(shape, dtype))
            zero_shapes.append((shape, dtype))
    n_params = len(in_names)
    all_names = in_names + out_names + ([pname] if pname else [])

    def _body(*args):
        operands = list(args)
        if pname is not None:
            operands.append(bass2jax.partition_id_tensor())
        outs = bass2jax._bass_exec_p.bind(
            *operands, out_avals=tuple(out_avals), in_names=tuple(all_names),
            out_names=tuple(out_names), lowering_input_output_aliases=(),
            sim_require_finite=True, sim_require_nnan=True, nc=nc)
        return tuple(outs)

    devices = jax.devices()[:N_CORES]
    mesh = Mesh(np.asarray(devices), ("core",))
    n_outs = len(out_names)
    in_specs = (PartitionSpec("core"),) * (n_params + n_outs)
    out_specs = (PartitionSpec("core"),) * n_outs
    donate = tuple(range(n_params, n_params + n_outs))
    sharded = jax.jit(
        shard_map(_body, mesh=mesh, in_specs=in_specs, out_specs=out_specs,
                  check_rep=False),
        donate_argnums=donate, keep_unused=True)

    def run(in_maps):
        concat_in = [
            np.concatenate([np.asarray(in_maps[c][nm]) for c in range(N_CORES)],
                           axis=0)
            for nm in in_names]
        concat_zeros = [np.zeros((N_CORES * s[0], *s[1:]), d)
                        for (s, d) in zero_shapes]
        out_arrs = sharded(*concat_in, *concat_zeros)
        return [
            {nm: np.asarray(out_arrs[i]).reshape(N_CORES, *out_avals[i].shape)[c]
             for i, nm in enumerate(out_names)}
            for c in range(N_CORES)]

    return run


def _get_runner():
    if "run" not in _STATE:
        nc = _build_nc()
        run = _make_runner(nc)
        # warm-up: triggers neuronx-cc (disk-cached) + device program load
        z = [{"xtb": np.zeros((INF, NPC), np.float16),
              "srcp": np.zeros((P, T * CH), np.uint16),
              "dlob": np.zeros((P, T * CH), np.float16),
              "batchb": np.zeros((P, T), np.float16),
              "win": np.zeros((4, 2 * INF, HID), np.float16),
              "wconv": np.zeros((N_CONV, 2, GRID * HID, HID), np.float16),
              "wro": np.zeros((P, 2 * HID + 1), np.float32)}
             for _ in range(N_CORES)]
        run(z)
        _STATE["run"] = run
    return _STATE["run"]


# ======================= host preprocessing =======================

def _preprocess(x, edge_index, batch, W_in, W_conv, W_out, b_out):
    src = np.ascontiguousarray(edge_index[0]).astype(np.int32)
    dst = np.ascontiguousarray(edge_index[1]).astype(np.int32)

    hi = (dst >> 7).astype(np.int16)
    perm = np.argsort(hi, kind="stable")
    src_s = src[perm]
    dst_s = dst[perm]
    n_buckets = N_CORES * T
    counts = np.bincount(hi, minlength=n_buckets).astype(np.int64)
    if counts.max() > CH * P:
        raise ValueError("bucket overflow")

    CAP = CH * P
    mask = np.arange(CAP)[None, :] < counts[:, None]
    srcpad = np.zeros((n_buckets, CAP), np.uint16)
    srcpad[mask] = src_s.astype(np.uint16)
    dlopad = np.full((n_buckets, CAP), 255, np.int16)
    dlopad[mask] = (dst_s & 127).astype(np.int16)
    srcpad = srcpad.reshape(n_buckets, CH, P)
    dlopad = dlopad.reshape(n_buckets, CH, P)

    batch_np = np.ascontiguousarray(batch).astype(np.int32)
    x32 = np.ascontiguousarray(x).astype(np.float32)
    n_nodes = x32.shape[0]

    KIN = 2 * INF
    KCV = GRID * HID
    W_in = np.asarray(W_in, np.float32)
    W_conv = np.asarray(W_conv, np.float32)
    W_out = np.asarray(W_out, np.float32)
    b_out = np.asarray(b_out, np.float32)
    win = np.zeros((4, KIN, HID), np.float16)
    for kk in range(GRID):
        blk, off = divmod(kk, 2)
        win[blk, off * INF:(off + 1) * INF, :] = W_in[0, :, :, kk].T
        win[2 + blk, off * INF:(off + 1) * INF, :] = W_in[1, :, :, kk].T
    wconv = np.zeros((N_CONV, 2, KCV, HID), np.float16)
    for l in range(N_CONV):
        for kk in range(GRID):
            wconv[l, 0, kk * HID:(kk + 1) * HID, :] = W_conv[l, 0, :, :, kk].T
            wconv[l, 1, kk * HID:(kk + 1) * HID, :] = W_conv[l, 1, :, :, kk].T
    wro = np.zeros((P, 2 * HID + 1), np.float32)
    wro[:, 0:HID] = W_out[0, 0, :, 0][None, :]
    wro[:, HID:2 * HID] = W_out[1, 0, :, 0][None, :]
    wro[:, 2 * HID] = b_out.ravel()[0]

    in_maps = []
    for c in range(N_CORES):
        lo = c * NPC
        xs = np.zeros((NPC, INF), np.float32)
        n_real = max(0, min(lo + NPC, n_nodes) - lo)
        if n_real > 0:
            xs[:n_real] = x32[lo:lo + n_real]
        xtb = np.ascontiguousarray(xs.T).astype(np.float16)

        bt = np.full(NPC, 255, np.int32)
        if n_real > 0:
            bt[:n_real] = batch_np[lo:lo + n_real]
        batchb = np.ascontiguousarray(bt.reshape(T, P).T).astype(np.float16)

        sp = srcpad[c * T:(c + 1) * T]
        dp = dlopad[c * T:(c + 1) * T]
        srcp = np.ascontiguousarray(sp.transpose(2, 0, 1).reshape(P, T * CH))
        dlob = np.ascontiguousarray(
            dp.transpose(2, 0, 1).reshape(P, T * CH)).astype(np.float16)

        in_maps.append({"xtb": xtb, "srcp": srcp, "dlob": dlob,
                        "batchb": batchb, "win": win, "wconv": wconv,
                        "wro": wro})
    return in_maps


# ======================= NumPy fallback =======================

def _kan_np(x, W, bias=None):
    g = W.shape[-1]
    k = np.arange(1, g + 1, dtype=np.float32)
    arg = x[:, :, None] * k
    B = x.shape[0]
    co = np.cos(arg).reshape(B, -1)
    si = np.sin(arg).reshape(B, -1)
    out_dim = W.shape[1]
    y = co @ W[0].reshape(out_dim, -1).T + si @ W[1].reshape(out_dim, -1).T
    if bias is not None:
        y = y + bias
    return y.astype(np.float32)


def _fallback(x, edge_index, batch, W_in, W_conv, W_out, b_out):
    x = np.asarray(x, np.float32)
    src = np.asarray(edge_index[0]).astype(np.int64)
    dst = np.asarray(edge_index[1]).astype(np.int64)
    bat = np.asarray(batch).astype(np.int64)
    W_in = np.asarray(W_in, np.float32)
    W_conv = np.asarray(W_conv, np.float32)
    W_out = np.asarray(W_out, np.float32)
    b_out = np.asarray(b_out, np.float32)
    n_nodes = x.shape[0]

    order = np.argsort(dst, kind="stable")
    dst_s = dst[order]
    src_s = src[order]
    seg_starts = np.searchsorted(dst_s, np.arange(n_nodes + 1))

    h = _kan_np(x, W_in)
    for l in range(N_CONV):
        msg = _kan_np(h, W_conv[l])
        gathered = msg[src_s]
        csum = np.concatenate(
            [np.zeros((1, h.shape[1]), np.float64),
             np.cumsum(gathered, axis=0, dtype=np.float64)], axis=0)
        m = (csum[seg_starts[1:]] - csum[seg_starts[:-1]]).astype(np.float32)
        z = m + h
        h = np.where(z >= 0, z, NEG_SLOPE * z).astype(np.float32)

    sums = np.zeros((N_GRAPHS, h.shape[1]), np.float64)
    np.add.at(sums, bat, h.astype(np.float64))
    counts = np.bincount(bat, minlength=N_GRAPHS).astype(np.float32)
    y = (sums.astype(np.float32)) / np.maximum(counts, 1.0)[:, None]
    out = _kan_np(y, W_out, b_out)
    return (1.0 / (1.0 + np.exp(-out))).astype(np.float32)


# ======================= entry point =======================

def kernel(x, edge_index, batch, W_in, W_conv, W_out, b_out):
    try:
        run = _get_runner()
        in_maps = _preprocess(x, edge_index, batch, W_in, W_conv, W_out, b_out)
        res = run(in_maps)
        out = np.asarray(res[0]["out"], np.float32)
        if out.shape != (N_GRAPHS, 1):
            raise ValueError("bad output shape")
        if not np.all(np.isfinite(out)) or out.min() < 0.0 or out.max() > 1.0:
            raise ValueError("output sanity check failed")
        if not np.array_equal(out, np.asarray(res[N_CORES - 1]["out"])):
            raise ValueError("cross-core mismatch")
        return out
    except Exception:
        return _fallback(x, edge_index, batch, W_in, W_conv, W_out, b_out)


# Build + compile + warm up at import so kernel() pays only per-input costs.
try:
    _get_runner()
except Exception:
    _STATE.pop("run", None)


# revision 3
# speedup vs baseline: 3.2593x; 1.0065x over previous
"""KA-GNN (Fourier-KAN message passing) on 8 Trainium2 NeuronCores via Bass.

Strategy (sharding_hint: partition nodes/edges across cores, replicate small
Fourier coefficients, collective only at readout):
 - nodes are split into 128-row tiles; each core owns T=49 consecutive tiles
   (dst buckets) -> node/edge partition across 8 cores.
 - per conv layer each core computes Fourier-KAN messages for its own nodes
   (Chebyshev recurrence on-device: cos/sin(k*h), k=1..4), writes a fp16 msg
   table slice to DRAM, AllGather -> full table.
 - scatter-add aggregation: edges bucketed by dst tile (host-side O(E) radix
   grouping), per 128-edge chunk one indirect-DMA row gather + one-hot
   selection matrix (is_equal vs iota) matmul accumulated in PSUM.
 - mean-pool partial sums per core -> AllReduce -> replicated KAN readout.

Import-time: trace + neuronx-cc compile (disk-cached) + warm-up execution.
kernel() call: host preprocessing + one sharded device invocation.
Any failure falls back to a pure-NumPy implementation.
"""

import numpy as np

P = 128
N_CORES = 8
T = 49                 # node tiles (dst buckets) per core
CH = 18                # max 128-edge chunks per bucket
INF = 64
HID = 32
GRID = 4
N_CONV = 2
NCH = 6                # trig free-dim chunking
N_NODES = 50000
N_GRAPHS = 128
NEG_SLOPE = 0.01
NPC = T * P
NPAD = NPC * N_CORES

MAGIC = 1.5 * 2.0**23
INV2PI = float(1.0 / (2.0 * np.pi))
TWOPI = float(2.0 * np.pi)

_STATE = {}


# ======================= device program =======================

def _build_nc():
    from contextlib import ExitStack
    import concourse.bass as bass
    import concourse.bacc as bacc
    import concourse.mybir as mybir
    import concourse.tile as tile
    from concourse.masks import make_identity

    F32 = mybir.dt.float32
    F16 = mybir.dt.float16
    I32 = mybir.dt.int32
    U16 = mybir.dt.uint16
    A = mybir.AluOpType
    ACT = mybir.ActivationFunctionType

    KIN = 2 * INF
    KCV = GRID * HID

    nc = bacc.Bacc("TRN2", target_bir_lowering=False, debug=False,
                   num_devices=N_CORES)

    xtb_d = nc.dram_tensor("xtb", [INF, NPC], F16, kind="ExternalInput").ap()
    srcp_d = nc.dram_tensor("srcp", [P, T * CH], U16, kind="ExternalInput").ap()
    dlob_d = nc.dram_tensor("dlob", [P, T * CH], F16, kind="ExternalInput").ap()
    batch_d = nc.dram_tensor("batchb", [P, T], F16, kind="ExternalInput").ap()
    win_d = nc.dram_tensor("win", [4, KIN, HID], F16, kind="ExternalInput").ap()
    wconv_d = nc.dram_tensor("wconv", [N_CONV, 2, KCV, HID], F16,
                             kind="ExternalInput").ap()
    wro_d = nc.dram_tensor("wro", [P, 2 * HID + 1], F32, kind="ExternalInput").ap()
    out_d = nc.dram_tensor("out", [P, 1], F32, kind="ExternalOutput").ap()

    replica = [list(range(N_CORES))]
    CHW = (T + NCH - 1) // NCH

    with tile.TileContext(nc) as tc, ExitStack() as ctx:
        sb = ctx.enter_context(tc.tile_pool(name="sb", bufs=1))
        tg = ctx.enter_context(tc.tile_pool(name="tg", bufs=1))
        sbg = ctx.enter_context(tc.tile_pool(name="sbg", bufs=4))
        sel_p = ctx.enter_context(tc.tile_pool(name="selp", bufs=2))
        pst = ctx.enter_context(tc.tile_pool(name="pst", bufs=2, space="PSUM"))
        psa = ctx.enter_context(tc.tile_pool(name="psa", bufs=2, space="PSUM"))
        dr = ctx.enter_context(tc.tile_pool(name="dr", bufs=1, space="DRAM"))

        # ---------------- setup ----------------
        xtb = sb.tile([INF, NPC], F16)
        nc.sync.dma_start(out=xtb[:], in_=xtb_d[:])
        srcp_u = sb.tile([P, T * CH], U16)
        nc.sync.dma_start(out=srcp_u[:], in_=srcp_d[:])
        srci = sb.tile([P, T * CH], I32)
        nc.vector.tensor_copy(srci[:], srcp_u[:])
        dlo = sb.tile([P, T * CH], F16)
        nc.sync.dma_start(out=dlo[:], in_=dlob_d[:])
        batchb = sb.tile([P, T], F16)
        nc.sync.dma_start(out=batchb[:], in_=batch_d[:])

        win_b = sb.tile([P, 4 * HID], F16)
        for b in range(4):
            nc.sync.dma_start(out=win_b[:KIN, b * HID:(b + 1) * HID], in_=win_d[b])
        wconv_b = sb.tile([P, N_CONV * 2 * HID], F16)
        for l in range(N_CONV):
            for s in range(2):
                c0 = (l * 2 + s) * HID
                nc.sync.dma_start(out=wconv_b[:KCV, c0:c0 + HID], in_=wconv_d[l, s])
        wro = sb.tile([P, 2 * HID + 1], F32)
        nc.sync.dma_start(out=wro[:], in_=wro_d[:])

        iota_i = sb.tile([P, P], I32)
        nc.gpsimd.iota(iota_i[:], pattern=[[1, P]], base=0, channel_multiplier=0)
        iota_h = sb.tile([P, P], F16)
        nc.vector.tensor_copy(iota_h[:], iota_i[:])
        ident = sb.tile([P, P], F32)
        make_identity(nc, ident[:])

        locl = [dr.tile([NPC, HID], F16, name=f"locl{i}") for i in range(N_CONV)]
        tbl = [dr.tile([NPAD, HID], F16, name=f"tbl{i}") for i in range(N_CONV)]
        arin = dr.tile([P, HID + 1], F32)
        arout = dr.tile([P, HID + 1], F32)

        h_buf_a = sb.tile([P, T * HID], F32, tag="hbuf_a")
        h_buf_b = sb.tile([P, T * HID], F32, tag="hbuf_b")
        TcA = sb.tile([P, NPC], F16, tag="TcA")
        TcB = sb.tile([P, NPC], F16, tag="TcB")
        TsA = sb.tile([P, NPC], F16, tag="TsA")
        TsB = sb.tile([P, NPC], F16, tag="TsB")

        def chebyshev(src, rows, cols, col0, dst_of):
            u = tg.tile([rows, CHW * P], F32, tag="ch_u")
            r = tg.tile([rows, CHW * P], F32, tag="ch_r")
            nc.vector.tensor_scalar(out=r[:rows, :cols], in0=src, scalar1=INV2PI,
                                    scalar2=MAGIC, op0=A.mult, op1=A.add)
            nc.vector.tensor_scalar(out=r[:rows, :cols], in0=r[:rows, :cols],
                                    scalar1=MAGIC, op0=A.subtract, scalar2=None)
            nc.vector.tensor_scalar(out=r[:rows, :cols], in0=r[:rows, :cols],
                                    scalar1=-TWOPI, op0=A.mult, scalar2=None)
            nc.vector.tensor_tensor(out=u[:rows, :cols], in0=src,
                                    in1=r[:rows, :cols], op=A.add)
            s1 = tg.tile([rows, CHW * P], F32, tag="ch_s1")
            chh = tg.tile([rows, CHW * P], F32, tag="ch_ch")
            c1 = tg.tile([rows, CHW * P], F32, tag="ch_c1")
            nc.scalar.activation(s1[:rows, :cols], u[:rows, :cols], ACT.Sin)
            nc.scalar.activation(chh[:rows, :cols], u[:rows, :cols], ACT.Sin,
                                 scale=0.5)
            nc.vector.tensor_tensor(out=c1[:rows, :cols], in0=chh[:rows, :cols],
                                    in1=chh[:rows, :cols], op=A.mult)
            nc.vector.tensor_scalar(out=c1[:rows, :cols], in0=c1[:rows, :cols],
                                    scalar1=-2.0, scalar2=1.0, op0=A.mult,
                                    op1=A.add)
            twoc = tg.tile([rows, CHW * P], F32, tag="ch_twoc")
            nc.vector.tensor_scalar(out=twoc[:rows, :cols], in0=c1[:rows, :cols],
                                    scalar1=2.0, op0=A.mult, scalar2=None)
            cs = {1: (c1, s1)}
            prev_c, prev_s = c1, s1
            pprev_c, pprev_s = None, None
            for k in range(2, GRID + 1):
                ck = tg.tile([rows, CHW * P], F32, tag=f"ch_c{k}")
                sk = tg.tile([rows, CHW * P], F32, tag=f"ch_s{k}")
                nc.vector.tensor_tensor(out=ck[:rows, :cols],
                                        in0=twoc[:rows, :cols],
                                        in1=prev_c[:rows, :cols], op=A.mult)
                nc.vector.tensor_tensor(out=sk[:rows, :cols],
                                        in0=twoc[:rows, :cols],
                                        in1=prev_s[:rows, :cols], op=A.mult)
                if k == 2:
                    nc.vector.tensor_scalar(out=ck[:rows, :cols],
                                            in0=ck[:rows, :cols], scalar1=1.0,
                                            op0=A.subtract, scalar2=None)
                else:
                    nc.vector.tensor_tensor(out=ck[:rows, :cols],
                                            in0=ck[:rows, :cols],
                                            in1=pprev_c[:rows, :cols],
                                            op=A.subtract)
                    nc.vector.tensor_tensor(out=sk[:rows, :cols],
                                            in0=sk[:rows, :cols],
                                            in1=pprev_s[:rows, :cols],
                                            op=A.subtract)
                cs[k] = (ck, sk)
                pprev_c, pprev_s = prev_c, prev_s
                prev_c, prev_s = ck, sk
            for k in range(1, GRID + 1):
                (ctile, r0), (stile, r1) = dst_of(k)
                ckf, skf = cs[k]
                nc.vector.tensor_copy(ctile[r0:r0 + rows, col0:col0 + cols],
                                      ckf[:rows, :cols])
                nc.vector.tensor_copy(stile[r1:r1 + rows, col0:col0 + cols],
                                      skf[:rows, :cols])

        # ---------- input layer ----------
        def in_dst(k):
            if k <= 2:
                return (TcA, (k - 1) * INF), (TsA, (k - 1) * INF)
            return (TcB, (k - 3) * INF), (TsB, (k - 3) * INF)

        for c0 in range(0, T, CHW):
            c1_ = min(c0 + CHW, T)
            cols = (c1_ - c0) * P
            chebyshev(xtb[:INF, c0 * P:c0 * P + cols], INF, cols, c0 * P, in_dst)

        h_cur, h_nxt = h_buf_a, h_buf_b
        for t in range(T):
            ph = psa.tile([P, HID], F32, tag="ps_acc")
            blocks = [(TcA, 0), (TcB, 1), (TsA, 2), (TsB, 3)]
            for i, (tt, b) in enumerate(blocks):
                nc.tensor.matmul(out=ph[:], lhsT=tt[:KIN, t * P:(t + 1) * P],
                                 rhs=win_b[:KIN, b * HID:(b + 1) * HID],
                                 start=(i == 0), stop=(i == 3))
            nc.vector.tensor_copy(h_cur[:, t * HID:(t + 1) * HID], ph[:])

        # ---------- conv layers ----------
        def conv_dst(k):
            return (TcA, (k - 1) * HID), (TcB, (k - 1) * HID)

        for l in range(N_CONV):
            for c0 in range(0, T, CHW):
                c1_ = min(c0 + CHW, T)
                cols = (c1_ - c0) * P
                h_T = tg.tile([HID, CHW * P], F32, tag="h_T")
                for t in range(c0, c1_):
                    pt = pst.tile([HID, P], F32, tag="ps_tr")
                    nc.tensor.transpose(out=pt[:],
                                        in_=h_cur[:, t * HID:(t + 1) * HID],
                                        identity=ident[:])
                    nc.vector.tensor_copy(
                        h_T[:, (t - c0) * P:(t - c0 + 1) * P], pt[:])
                chebyshev(h_T[:HID, :cols], HID, cols, c0 * P, conv_dst)

            msg_sb = sb.tile([P, T * HID], F16, tag="msg_sb")
            for t in range(T):
                pm = psa.tile([P, HID], F32, tag="ps_acc")
                nc.tensor.matmul(out=pm[:], lhsT=TcA[:KCV, t * P:(t + 1) * P],
                                 rhs=wconv_b[:KCV, (l * 2) * HID:(l * 2 + 1) * HID],
                                 start=True, stop=False)
                nc.tensor.matmul(out=pm[:], lhsT=TcB[:KCV, t * P:(t + 1) * P],
                                 rhs=wconv_b[:KCV, (l * 2 + 1) * HID:(l * 2 + 2) * HID],
                                 start=False, stop=True)
                nc.vector.tensor_copy(msg_sb[:, t * HID:(t + 1) * HID], pm[:])
            nc.gpsimd.dma_start(
                out=locl[l][:].rearrange("(t p) f -> p t f", p=P),
                in_=msg_sb[:].rearrange("p (t f) -> p t f", t=T))
            nc.gpsimd.collective_compute(
                "AllGather", A.bypass, replica_groups=replica,
                ins=[locl[l].opt()], outs=[tbl[l].opt()])

            for b in range(T):
                sel = sel_p.tile([P, CH * P], F16, tag="sel")
                nc.vector.tensor_tensor(
                    out=sel[:].rearrange("p (c q) -> p c q", c=CH),
                    in0=dlo[:, b * CH:(b + 1) * CH]
                        .rearrange("p (c o) -> p c o", o=1)
                        .to_broadcast([P, CH, P]),
                    in1=iota_h[:].rearrange("p (o q) -> p o q", o=1)
                        .to_broadcast([P, CH, P]),
                    op=A.is_equal)
                pagg = psa.tile([P, HID], F32, tag="ps_agg")
                for j in range(CH):
                    g_t = sbg.tile([P, HID], F16, tag="gath")
                    nc.gpsimd.indirect_dma_start(
                        out=g_t[:], out_offset=None, in_=tbl[l][:],
                        in_offset=bass.IndirectOffsetOnAxis(
                            ap=srci[:, b * CH + j:b * CH + j + 1], axis=0))
                    nc.tensor.matmul(out=pagg[:],
                                     lhsT=sel[:, j * P:(j + 1) * P],
                                     rhs=g_t[:], start=(j == 0),
                                     stop=(j == CH - 1))
                z = sbg.tile([P, HID], F32, tag="zt")
                nc.vector.tensor_tensor(out=z[:], in0=pagg[:],
                                        in1=h_cur[:, b * HID:(b + 1) * HID],
                                        op=A.add)
                zs = sbg.tile([P, HID], F32, tag="zs")
                nc.vector.tensor_scalar(out=zs[:], in0=z[:], scalar1=NEG_SLOPE,
                                        op0=A.mult, scalar2=None)
                nc.vector.tensor_tensor(out=h_nxt[:, b * HID:(b + 1) * HID],
                                        in0=z[:], in1=zs[:], op=A.max)
            h_cur, h_nxt = h_nxt, h_cur

        # ---------- pooling ----------
        h2b = sb.tile([P, T * (HID + 1)], F16, tag="h2b")
        for t in range(T):
            nc.vector.tensor_copy(
                h2b[:, t * (HID + 1):t * (HID + 1) + HID],
                h_cur[:, t * HID:(t + 1) * HID])
            nc.gpsimd.memset(h2b[:, t * (HID + 1) + HID:(t + 1) * (HID + 1)], 1.0)
        ppool = psa.tile([P, HID + 1], F32, tag="ps_acc")
        for t in range(T):
            selB = sbg.tile([P, P], F16, tag="selB")
            nc.vector.tensor_tensor(
                out=selB[:],
                in0=batchb[:, t:t + 1].to_broadcast([P, P]),
                in1=iota_h[:], op=A.is_equal)
            nc.tensor.matmul(out=ppool[:], lhsT=selB[:],
                             rhs=h2b[:, t * (HID + 1):(t + 1) * (HID + 1)],
                             start=(t == 0), stop=(t == T - 1))
        pool_sb = sb.tile([P, HID + 1], F32)
        nc.vector.tensor_copy(pool_sb[:], ppool[:])
        nc.gpsimd.dma_start(out=arin[:], in_=pool_sb[:])
        nc.gpsimd.collective_compute(
            "AllReduce", A.add, replica_groups=replica,
            ins=[arin.opt()], outs=[arout.opt()])
        sums = sb.tile([P, HID + 1], F32)
        nc.gpsimd.dma_start(out=sums[:], in_=arout[:])

        cnt = sb.tile([P, 1], F32)
        nc.vector.tensor_scalar(out=cnt[:], in0=sums[:, HID:HID + 1],
                                scalar1=1.0, op0=A.max, scalar2=None)
        rec = sb.tile([P, 1], F32)
        nc.vector.reciprocal(rec[:], cnt[:])
        y = sb.tile([P, HID], F32)
        nc.vector.tensor_tensor(out=y[:], in0=sums[:, :HID],
                                in1=rec[:].to_broadcast([P, HID]), op=A.mult)

        ur = sb.tile([P, HID], F32)
        rr_ = sb.tile([P, HID], F32)
        nc.vector.tensor_scalar(out=rr_[:], in0=y[:], scalar1=INV2PI,
                                scalar2=MAGIC, op0=A.mult, op1=A.add)
        nc.vector.tensor_scalar(out=rr_[:], in0=rr_[:], scalar1=MAGIC,
                                op0=A.subtract, scalar2=None)
        nc.vector.tensor_scalar(out=rr_[:], in0=rr_[:], scalar1=-TWOPI,
                                op0=A.mult, scalar2=None)
        nc.vector.tensor_tensor(out=ur[:], in0=y[:], in1=rr_[:], op=A.add)
        sy = sb.tile([P, HID], F32)
        chy = sb.tile([P, HID], F32)
        cy = sb.tile([P, HID], F32)
        nc.scalar.activation(sy[:], ur[:], ACT.Sin)
        nc.scalar.activation(chy[:], ur[:], ACT.Sin, scale=0.5)
        nc.vector.tensor_tensor(out=cy[:], in0=chy[:], in1=chy[:], op=A.mult)
        nc.vector.tensor_scalar(out=cy[:], in0=cy[:], scalar1=-2.0, scalar2=1.0,
                                op0=A.mult, op1=A.add)
        dotc = sb.tile([P, HID], F32)
        accc = sb.tile([P, 1], F32)
        nc.vector.tensor_tensor(out=dotc[:], in0=cy[:], in1=wro[:, 0:HID],
                                op=A.mult)
        nc.vector.tensor_reduce(out=accc[:], in_=dotc[:],
                                axis=mybir.AxisListType.X, op=A.add)
        dots = sb.tile([P, HID], F32)
        accs = sb.tile([P, 1], F32)
        nc.vector.tensor_tensor(out=dots[:], in0=sy[:], in1=wro[:, HID:2 * HID],
                                op=A.mult)
        nc.vector.tensor_reduce(out=accs[:], in_=dots[:],
                                axis=mybir.AxisListType.X, op=A.add)
        o_t = sb.tile([P, 1], F32)
        nc.vector.tensor_tensor(out=o_t[:], in0=accc[:], in1=accs[:], op=A.add)
        nc.vector.tensor_tensor(out=o_t[:], in0=o_t[:],
                                in1=wro[:, 2 * HID:2 * HID + 1], op=A.add)
        res = sb.tile([P, 1], F32)
        nc.scalar.activation(res[:], o_t[:], ACT.Sigmoid)
        nc.sync.dma_start(out=out_d[:], in_=res[:])

    nc.compile()
    return nc


def _make_runner(nc):
    import jax
    from jax.sharding import Mesh, PartitionSpec
    from jax.experimental.shard_map import shard_map
    import concourse.mybir as mybir
    from concourse import bass2jax

    bass2jax.install_neuronx_cc_hook()
    pname = nc.partition_id_tensor.name if nc.partition_id_tensor else None
    in_names, out_names, out_avals, zero_shapes = [], [], [], []
    for alloc in nc.m.functions[0].allocations:
        if not isinstance(alloc, mybir.MemoryLocationSet):
            continue
        name = alloc.memorylocations[0].name
        if alloc.kind == "ExternalInput":
            if name != pname:
                in_names.append(name)
        elif alloc.kind == "ExternalOutput":
            out_names.append(name)
            shape = tuple(alloc.tensor_shape)
            dtype = mybir.dt.np(alloc.dtype)
            out_avals.append(jax.core.ShapedArray(shape, dtype))
            zero_shapes.append((shape, dtype))
    n_params = len(in_names)
    all_names = in_names + out_names + ([pname] if pname else [])

    def _body(*args):
        operands = list(args)
        if pname is not None:
            operands.append(bass2jax.partition_id_tensor())
        outs = bass2jax._bass_exec_p.bind(
            *operands, out_avals=tuple(out_avals), in_names=tuple(all_names),
            out_names=tuple(out_names), lowering_input_output_aliases=(),
            sim_require_finite=True, sim_require_nnan=True, nc=nc)
        return tuple(outs)

    devices = jax.devices()[:N_CORES]
    mesh = Mesh(np.asarray(devices), ("core",))
    n_outs = len(out_names)
    in_specs = (PartitionSpec("core"),) * (n_params + n_outs)
    out_specs = (PartitionSpec("core"),) * n_outs
    donate = tuple(range(n_params, n_params + n_outs))
    sharded = jax.jit(
        shard_map(_body, mesh=mesh, in_specs=in_specs, out_specs=out_specs,
                  check_rep=False),
        donate_argnums=donate, keep_unused=True)

    def run(in_maps):
        concat_in = [
            np.concatenate([np.asarray(in_maps[c][nm]) for c in range(N_CORES)],
                           axis=0)
            for nm in in_names]
        concat_zeros = [np.zeros((N_CORES * s[0], *s[1:]), d)
                        for (s, d) in zero_shapes]
        out_arrs = sharded(*concat_in, *concat_zeros)
        return [
            {nm: np.asarray(out_arrs[i]).reshape(N_CORES, *out_avals[i].shape)[c]
             for i, nm in enumerate(out_names)}
            for c in range(N_CORES)]

    return run


def _warmup(run):
    import jax
    z = [{"xtb": np.zeros((INF, NPC), np.float16),
          "srcp": np.zeros((P, T * CH), np.uint16),
          "dlob": np.zeros((P, T * CH), np.float16),
          "batchb": np.zeros((P, T), np.float16),
          "win": np.zeros((4, 2 * INF, HID), np.float16),
          "wconv": np.zeros((N_CONV, 2, GRID * HID, HID), np.float16),
          "wro": np.zeros((P, 2 * HID + 1), np.float32)}
         for _ in range(N_CORES)]
    run(z)
    jax.effects_barrier()


def _get_runner():
    if _STATE.get("dead"):
        raise RuntimeError("device path disabled")
    if "run" not in _STATE:
        nc = _build_nc()
        run = _make_runner(nc)
        # warm-up: triggers neuronx-cc (disk-cached) + device program load.
        # A failure here usually means the device was left in a bad state by
        # a previous process; that first failed attempt resets it, so retry
        # once before giving up on the device path.
        try:
            _warmup(run)
        except Exception:
            try:
                _warmup(run)
            except Exception:
                _STATE["dead"] = True
                raise
        _STATE["run"] = run
    return _STATE["run"]


# ======================= host preprocessing =======================

def _preprocess(x, edge_index, batch, W_in, W_conv, W_out, b_out):
    src = np.ascontiguousarray(edge_index[0]).astype(np.int32)
    dst = np.ascontiguousarray(edge_index[1]).astype(np.int32)

    hi = (dst >> 7).astype(np.int16)
    perm = np.argsort(hi, kind="stable")
    src_s = src[perm]
    dst_s = dst[perm]
    n_buckets = N_CORES * T
    counts = np.bincount(hi, minlength=n_buckets).astype(np.int64)
    if counts.max() > CH * P:
        raise ValueError("bucket overflow")

    CAP = CH * P
    mask = np.arange(CAP)[None, :] < counts[:, None]
    srcpad = np.zeros((n_buckets, CAP), np.uint16)
    srcpad[mask] = src_s.astype(np.uint16)
    dlopad = np.full((n_buckets, CAP), 255, np.int16)
    dlopad[mask] = (dst_s & 127).astype(np.int16)
    srcpad = srcpad.reshape(n_buckets, CH, P)
    dlopad = dlopad.reshape(n_buckets, CH, P)

    batch_np = np.ascontiguousarray(batch).astype(np.int32)
    x32 = np.ascontiguousarray(x).astype(np.float32)
    n_nodes = x32.shape[0]

    KIN = 2 * INF
    KCV = GRID * HID
    W_in = np.asarray(W_in, np.float32)
    W_conv = np.asarray(W_conv, np.float32)
    W_out = np.asarray(W_out, np.float32)
    b_out = np.asarray(b_out, np.float32)
    win = np.zeros((4, KIN, HID), np.float16)
    for kk in range(GRID):
        blk, off = divmod(kk, 2)
        win[blk, off * INF:(off + 1) * INF, :] = W_in[0, :, :, kk].T
        win[2 + blk, off * INF:(off + 1) * INF, :] = W_in[1, :, :, kk].T
    wconv = np.zeros((N_CONV, 2, KCV, HID), np.float16)
    for l in range(N_CONV):
        for kk in range(GRID):
            wconv[l, 0, kk * HID:(kk + 1) * HID, :] = W_conv[l, 0, :, :, kk].T
            wconv[l, 1, kk * HID:(kk + 1) * HID, :] = W_conv[l, 1, :, :, kk].T
    wro = np.zeros((P, 2 * HID + 1), np.float32)
    wro[:, 0:HID] = W_out[0, 0, :, 0][None, :]
    wro[:, HID:2 * HID] = W_out[1, 0, :, 0][None, :]
    wro[:, 2 * HID] = b_out.ravel()[0]

    in_maps = []
    for c in range(N_CORES):
        lo = c * NPC
        xs = np.zeros((NPC, INF), np.float32)
        n_real = max(0, min(lo + NPC, n_nodes) - lo)
        if n_real > 0:
            xs[:n_real] = x32[lo:lo + n_real]
        xtb = np.ascontiguousarray(xs.T).astype(np.float16)

        bt = np.full(NPC, 255, np.int32)
        if n_real > 0:
            bt[:n_real] = batch_np[lo:lo + n_real]
        batchb = np.ascontiguousarray(bt.reshape(T, P).T).astype(np.float16)

        sp = srcpad[c * T:(c + 1) * T]
        dp = dlopad[c * T:(c + 1) * T]
        srcp = np.ascontiguousarray(sp.transpose(2, 0, 1).reshape(P, T * CH))
        dlob = np.ascontiguousarray(
            dp.transpose(2, 0, 1).reshape(P, T * CH)).astype(np.float16)

        in_maps.append({"xtb": xtb, "srcp": srcp, "dlob": dlob,
                        "batchb": batchb, "win": win, "wconv": wconv,
                        "wro": wro})
    return in_maps


# ======================= NumPy fallback =======================

def _kan_np(x, W, bias=None):
    g = W.shape[-1]
    k = np.arange(1, g + 1, dtype=np.float32)
    arg = x[:, :, None] * k
    B = x.shape[0]
    co = np.cos(arg).reshape(B, -1)
    si = np.sin(arg).reshape(B, -1)
    out_dim = W.shape[1]
    y = co @ W[0].reshape(out_dim, -1).T + si @ W[1].reshape(out_dim, -1).T
    if bias is not None:
        y = y + bias
    return y.astype(np.float32)


def _fallback(x, edge_index, batch, W_in, W_conv, W_out, b_out):
    x = np.asarray(x, np.float32)
    src = np.asarray(edge_index[0]).astype(np.int64)
    dst = np.asarray(edge_index[1]).astype(np.int64)
    bat = np.asarray(batch).astype(np.int64)
    W_in = np.asarray(W_in, np.float32)
    W_conv = np.asarray(W_conv, np.float32)
    W_out = np.asarray(W_out, np.float32)
    b_out = np.asarray(b_out, np.float32)
    n_nodes = x.shape[0]

    order = np.argsort(dst, kind="stable")
    dst_s = dst[order]
    src_s = src[order]
    seg_starts = np.searchsorted(dst_s, np.arange(n_nodes + 1))

    h = _kan_np(x, W_in)
    for l in range(N_CONV):
        msg = _kan_np(h, W_conv[l])
        gathered = msg[src_s]
        csum = np.concatenate(
            [np.zeros((1, h.shape[1]), np.float64),
             np.cumsum(gathered, axis=0, dtype=np.float64)], axis=0)
        m = (csum[seg_starts[1:]] - csum[seg_starts[:-1]]).astype(np.float32)
        z = m + h
        h = np.where(z >= 0, z, NEG_SLOPE * z).astype(np.float32)

    sums = np.zeros((N_GRAPHS, h.shape[1]), np.float64)
    np.add.at(sums, bat, h.astype(np.float64))
    counts = np.bincount(bat, minlength=N_GRAPHS).astype(np.float32)
    y = (sums.astype(np.float32)) / np.maximum(counts, 1.0)[:, None]
    out = _kan_np(y, W_out, b_out)
    return (1.0 / (1.0 + np.exp(-out))).astype(np.float32)


# ======================= entry point =======================

def kernel(x, edge_index, batch, W_in, W_conv, W_out, b_out):
    try:
        import jax
        run = _get_runner()
        in_maps = _preprocess(x, edge_index, batch, W_in, W_conv, W_out, b_out)
        res = run(in_maps)
        out = np.asarray(res[0]["out"], np.float32)
        # Wait for full device-side completion (incl. kernel-tail barriers)
        # before returning: exiting the process mid-tail wedges the device.
        jax.effects_barrier()
        if out.shape != (N_GRAPHS, 1):
            raise ValueError("bad output shape")
        if not np.all(np.isfinite(out)) or out.min() < 0.0 or out.max() > 1.0:
            raise ValueError("output sanity check failed")
        if not np.array_equal(out, np.asarray(res[N_CORES - 1]["out"])):
            raise ValueError("cross-core mismatch")
        return out
    except Exception:
        return _fallback(x, edge_index, batch, W_in, W_conv, W_out, b_out)


# Build + compile + warm up at import so kernel() pays only per-input costs.
try:
    _get_runner()
except Exception:
    _STATE.pop("run", None)


# revision 5
# speedup vs baseline: 3.4135x; 1.0473x over previous
"""KA-GNN (Fourier-KAN message passing) on 8 Trainium2 NeuronCores via Bass.

Strategy (sharding_hint: partition nodes/edges across cores, replicate small
Fourier coefficients, collective only at readout):
 - nodes are split into 128-row tiles; each core owns T=49 consecutive tiles
   (dst buckets) -> node/edge partition across 8 cores.
 - per conv layer each core computes Fourier-KAN messages for its own nodes
   (Chebyshev recurrence on-device: cos/sin(k*h), k=1..4), writes a fp16 msg
   table slice to DRAM, AllGather -> full table.
 - scatter-add aggregation: edges bucketed by dst tile (host-side O(E) radix
   grouping), per 128-edge chunk one indirect-DMA row gather + one-hot
   selection matrix (is_equal vs iota) matmul accumulated in PSUM.
 - mean-pool partial sums per core -> AllReduce -> replicated KAN readout.

Import-time: trace + neuronx-cc compile (disk-cached) + warm-up execution.
kernel() call: host preprocessing + one sharded device invocation.
Any failure falls back to a pure-NumPy implementation.
"""

import numpy as np

P = 128
N_CORES = 8
T = 49                 # node tiles (dst buckets) per core
CH = 18                # max 128-edge chunks per bucket
INF = 64
HID = 32
GRID = 4
N_CONV = 2
NCH = 6                # trig free-dim chunking
N_NODES = 50000
N_GRAPHS = 128
NEG_SLOPE = 0.01
NPC = T * P
NPAD = NPC * N_CORES

MAGIC = 1.5 * 2.0**23
INV2PI = float(1.0 / (2.0 * np.pi))
TWOPI = float(2.0 * np.pi)

_STATE = {}


# ======================= device program =======================

def _build_nc():
    from contextlib import ExitStack
    import concourse.bass as bass
    import concourse.bacc as bacc
    import concourse.mybir as mybir
    import concourse.tile as tile
    from concourse.masks import make_identity

    F32 = mybir.dt.float32
    F16 = mybir.dt.float16
    I32 = mybir.dt.int32
    U16 = mybir.dt.uint16
    A = mybir.AluOpType
    ACT = mybir.ActivationFunctionType

    KIN = 2 * INF
    KCV = GRID * HID

    nc = bacc.Bacc("TRN2", target_bir_lowering=False, debug=False,
                   num_devices=N_CORES)

    xtb_d = nc.dram_tensor("xtb", [INF, NPC], F16, kind="ExternalInput").ap()
    srcp_d = nc.dram_tensor("srcp", [P, T * CH], U16, kind="ExternalInput").ap()
    dlob_d = nc.dram_tensor("dlob", [P, T * CH], F16, kind="ExternalInput").ap()
    batch_d = nc.dram_tensor("batchb", [P, T], F16, kind="ExternalInput").ap()
    win_d = nc.dram_tensor("win", [4, KIN, HID], F16, kind="ExternalInput").ap()
    wconv_d = nc.dram_tensor("wconv", [N_CONV, 2, KCV, HID], F16,
                             kind="ExternalInput").ap()
    wro_d = nc.dram_tensor("wro", [P, 2 * HID + 1], F32, kind="ExternalInput").ap()
    out_d = nc.dram_tensor("out", [P, 1], F32, kind="ExternalOutput").ap()

    replica = [list(range(N_CORES))]
    CHW = (T + NCH - 1) // NCH

    with tile.TileContext(nc) as tc, ExitStack() as ctx:
        sb = ctx.enter_context(tc.tile_pool(name="sb", bufs=1))
        tg = ctx.enter_context(tc.tile_pool(name="tg", bufs=1))
        sbg = ctx.enter_context(tc.tile_pool(name="sbg", bufs=4))
        sel_p = ctx.enter_context(tc.tile_pool(name="selp", bufs=2))
        pst = ctx.enter_context(tc.tile_pool(name="pst", bufs=2, space="PSUM"))
        psa = ctx.enter_context(tc.tile_pool(name="psa", bufs=2, space="PSUM"))
        dr = ctx.enter_context(tc.tile_pool(name="dr", bufs=1, space="DRAM"))

        # ---------------- setup ----------------
        xtb = sb.tile([INF, NPC], F16)
        nc.sync.dma_start(out=xtb[:], in_=xtb_d[:])
        srcp_u = sb.tile([P, T * CH], U16)
        nc.sync.dma_start(out=srcp_u[:], in_=srcp_d[:])
        srci = sb.tile([P, T * CH], I32)
        nc.vector.tensor_copy(srci[:], srcp_u[:])
        dlo = sb.tile([P, T * CH], F16)
        nc.sync.dma_start(out=dlo[:], in_=dlob_d[:])
        batchb = sb.tile([P, T], F16)
        nc.sync.dma_start(out=batchb[:], in_=batch_d[:])

        win_b = sb.tile([P, 4 * HID], F16)
        for b in range(4):
            nc.sync.dma_start(out=win_b[:KIN, b * HID:(b + 1) * HID], in_=win_d[b])
        wconv_b = sb.tile([P, N_CONV * 2 * HID], F16)
        for l in range(N_CONV):
            for s in range(2):
                c0 = (l * 2 + s) * HID
                nc.sync.dma_start(out=wconv_b[:KCV, c0:c0 + HID], in_=wconv_d[l, s])
        wro = sb.tile([P, 2 * HID + 1], F32)
        nc.sync.dma_start(out=wro[:], in_=wro_d[:])

        iota_i = sb.tile([P, P], I32)
        nc.gpsimd.iota(iota_i[:], pattern=[[1, P]], base=0, channel_multiplier=0)
        iota_h = sb.tile([P, P], F16)
        nc.vector.tensor_copy(iota_h[:], iota_i[:])
        ident = sb.tile([P, P], F32)
        make_identity(nc, ident[:])

        locl = [dr.tile([NPC, HID], F16, name=f"locl{i}") for i in range(N_CONV)]
        tbl = [dr.tile([NPAD, HID], F16, name=f"tbl{i}") for i in range(N_CONV)]
        arin = dr.tile([P, HID + 1], F32)
        arout = dr.tile([P, HID + 1], F32)

        h_buf_a = sb.tile([P, T * HID], F32, tag="hbuf_a")
        h_buf_b = sb.tile([P, T * HID], F32, tag="hbuf_b")
        TcA = sb.tile([P, NPC], F16, tag="TcA")
        TcB = sb.tile([P, NPC], F16, tag="TcB")
        TsA = sb.tile([P, NPC], F16, tag="TsA")
        TsB = sb.tile([P, NPC], F16, tag="TsB")

        def chebyshev(src, rows, cols, col0, dst_of):
            u = tg.tile([rows, CHW * P], F32, tag="ch_u")
            r = tg.tile([rows, CHW * P], F32, tag="ch_r")
            nc.vector.tensor_scalar(out=r[:rows, :cols], in0=src, scalar1=INV2PI,
                                    scalar2=MAGIC, op0=A.mult, op1=A.add)
            nc.vector.tensor_scalar(out=r[:rows, :cols], in0=r[:rows, :cols],
                                    scalar1=MAGIC, op0=A.subtract, scalar2=None)
            nc.vector.tensor_scalar(out=r[:rows, :cols], in0=r[:rows, :cols],
                                    scalar1=-TWOPI, op0=A.mult, scalar2=None)
            nc.vector.tensor_tensor(out=u[:rows, :cols], in0=src,
                                    in1=r[:rows, :cols], op=A.add)
            s1 = tg.tile([rows, CHW * P], F32, tag="ch_s1")
            chh = tg.tile([rows, CHW * P], F32, tag="ch_ch")
            c1 = tg.tile([rows, CHW * P], F32, tag="ch_c1")
            nc.scalar.activation(s1[:rows, :cols], u[:rows, :cols], ACT.Sin)
            nc.scalar.activation(chh[:rows, :cols], u[:rows, :cols], ACT.Sin,
                                 scale=0.5)
            nc.vector.tensor_tensor(out=c1[:rows, :cols], in0=chh[:rows, :cols],
                                    in1=chh[:rows, :cols], op=A.mult)
            nc.vector.tensor_scalar(out=c1[:rows, :cols], in0=c1[:rows, :cols],
                                    scalar1=-2.0, scalar2=1.0, op0=A.mult,
                                    op1=A.add)
            twoc = tg.tile([rows, CHW * P], F32, tag="ch_twoc")
            nc.vector.tensor_scalar(out=twoc[:rows, :cols], in0=c1[:rows, :cols],
                                    scalar1=2.0, op0=A.mult, scalar2=None)
            cs = {1: (c1, s1)}
            prev_c, prev_s = c1, s1
            pprev_c, pprev_s = None, None
            for k in range(2, GRID + 1):
                ck = tg.tile([rows, CHW * P], F32, tag=f"ch_c{k}")
                sk = tg.tile([rows, CHW * P], F32, tag=f"ch_s{k}")
                nc.vector.tensor_tensor(out=ck[:rows, :cols],
                                        in0=twoc[:rows, :cols],
                                        in1=prev_c[:rows, :cols], op=A.mult)
                nc.vector.tensor_tensor(out=sk[:rows, :cols],
                                        in0=twoc[:rows, :cols],
                                        in1=prev_s[:rows, :cols], op=A.mult)
                if k == 2:
                    nc.vector.tensor_scalar(out=ck[:rows, :cols],
                                            in0=ck[:rows, :cols], scalar1=1.0,
                                            op0=A.subtract, scalar2=None)
                else:
                    nc.vector.tensor_tensor(out=ck[:rows, :cols],
                                            in0=ck[:rows, :cols],
                                            in1=pprev_c[:rows, :cols],
                                            op=A.subtract)
                    nc.vector.tensor_tensor(out=sk[:rows, :cols],
                                            in0=sk[:rows, :cols],
                                            in1=pprev_s[:rows, :cols],
                                            op=A.subtract)
                cs[k] = (ck, sk)
                pprev_c, pprev_s = prev_c, prev_s
                prev_c, prev_s = ck, sk
            for k in range(1, GRID + 1):
                (ctile, r0), (stile, r1) = dst_of(k)
                ckf, skf = cs[k]
                nc.vector.tensor_copy(ctile[r0:r0 + rows, col0:col0 + cols],
                                      ckf[:rows, :cols])
                nc.vector.tensor_copy(stile[r1:r1 + rows, col0:col0 + cols],
                                      skf[:rows, :cols])

        # ---------- input layer ----------
        def in_dst(k):
            if k <= 2:
                return (TcA, (k - 1) * INF), (TsA, (k - 1) * INF)
            return (TcB, (k - 3) * INF), (TsB, (k - 3) * INF)

        for c0 in range(0, T, CHW):
            c1_ = min(c0 + CHW, T)
            cols = (c1_ - c0) * P
            chebyshev(xtb[:INF, c0 * P:c0 * P + cols], INF, cols, c0 * P, in_dst)

        h_cur, h_nxt = h_buf_a, h_buf_b
        for t in range(T):
            ph = psa.tile([P, HID], F32, tag="ps_acc")
            blocks = [(TcA, 0), (TcB, 1), (TsA, 2), (TsB, 3)]
            for i, (tt, b) in enumerate(blocks):
                nc.tensor.matmul(out=ph[:], lhsT=tt[:KIN, t * P:(t + 1) * P],
                                 rhs=win_b[:KIN, b * HID:(b + 1) * HID],
                                 start=(i == 0), stop=(i == 3))
            nc.vector.tensor_copy(h_cur[:, t * HID:(t + 1) * HID], ph[:])

        # ---------- conv layers ----------
        def conv_dst(k):
            return (TcA, (k - 1) * HID), (TcB, (k - 1) * HID)

        for l in range(N_CONV):
            for c0 in range(0, T, CHW):
                c1_ = min(c0 + CHW, T)
                cols = (c1_ - c0) * P
                h_T = tg.tile([HID, CHW * P], F32, tag="h_T")
                for t in range(c0, c1_):
                    pt = pst.tile([HID, P], F32, tag="ps_tr")
                    nc.tensor.transpose(out=pt[:],
                                        in_=h_cur[:, t * HID:(t + 1) * HID],
                                        identity=ident[:])
                    nc.vector.tensor_copy(
                        h_T[:, (t - c0) * P:(t - c0 + 1) * P], pt[:])
                chebyshev(h_T[:HID, :cols], HID, cols, c0 * P, conv_dst)

            msg_sb = sb.tile([P, T * HID], F16, tag="msg_sb")
            for t in range(T):
                pm = psa.tile([P, HID], F32, tag="ps_acc")
                nc.tensor.matmul(out=pm[:], lhsT=TcA[:KCV, t * P:(t + 1) * P],
                                 rhs=wconv_b[:KCV, (l * 2) * HID:(l * 2 + 1) * HID],
                                 start=True, stop=False)
                nc.tensor.matmul(out=pm[:], lhsT=TcB[:KCV, t * P:(t + 1) * P],
                                 rhs=wconv_b[:KCV, (l * 2 + 1) * HID:(l * 2 + 2) * HID],
                                 start=False, stop=True)
                nc.vector.tensor_copy(msg_sb[:, t * HID:(t + 1) * HID], pm[:])
            nc.gpsimd.dma_start(
                out=locl[l][:].rearrange("(t p) f -> p t f", p=P),
                in_=msg_sb[:].rearrange("p (t f) -> p t f", t=T))
            nc.gpsimd.collective_compute(
                "AllGather", A.bypass, replica_groups=replica,
                ins=[locl[l].opt()], outs=[tbl[l].opt()])

            for b in range(T):
                sel = sel_p.tile([P, CH * P], F16, tag="sel")
                nc.vector.tensor_tensor(
                    out=sel[:].rearrange("p (c q) -> p c q", c=CH),
                    in0=dlo[:, b * CH:(b + 1) * CH]
                        .rearrange("p (c o) -> p c o", o=1)
                        .to_broadcast([P, CH, P]),
                    in1=iota_h[:].rearrange("p (o q) -> p o q", o=1)
                        .to_broadcast([P, CH, P]),
                    op=A.is_equal)
                pagg = psa.tile([P, HID], F32, tag="ps_agg")
                for j in range(CH):
                    g_t = sbg.tile([P, HID], F16, tag="gath")
                    nc.gpsimd.indirect_dma_start(
                        out=g_t[:], out_offset=None, in_=tbl[l][:],
                        in_offset=bass.IndirectOffsetOnAxis(
                            ap=srci[:, b * CH + j:b * CH + j + 1], axis=0))
                    nc.tensor.matmul(out=pagg[:],
                                     lhsT=sel[:, j * P:(j + 1) * P],
                                     rhs=g_t[:], start=(j == 0),
                                     stop=(j == CH - 1))
                z = sbg.tile([P, HID], F32, tag="zt")
                nc.vector.tensor_tensor(out=z[:], in0=pagg[:],
                                        in1=h_cur[:, b * HID:(b + 1) * HID],
                                        op=A.add)
                zs = sbg.tile([P, HID], F32, tag="zs")
                nc.vector.tensor_scalar(out=zs[:], in0=z[:], scalar1=NEG_SLOPE,
                                        op0=A.mult, scalar2=None)
                nc.vector.tensor_tensor(out=h_nxt[:, b * HID:(b + 1) * HID],
                                        in0=z[:], in1=zs[:], op=A.max)
            h_cur, h_nxt = h_nxt, h_cur

        # ---------- pooling ----------
        h2b = sb.tile([P, T * (HID + 1)], F16, tag="h2b")
        for t in range(T):
            nc.vector.tensor_copy(
                h2b[:, t * (HID + 1):t * (HID + 1) + HID],
                h_cur[:, t * HID:(t + 1) * HID])
            nc.gpsimd.memset(h2b[:, t * (HID + 1) + HID:(t + 1) * (HID + 1)], 1.0)
        ppool = psa.tile([P, HID + 1], F32, tag="ps_acc")
        for t in range(T):
            selB = sbg.tile([P, P], F16, tag="selB")
            nc.vector.tensor_tensor(
                out=selB[:],
                in0=batchb[:, t:t + 1].to_broadcast([P, P]),
                in1=iota_h[:], op=A.is_equal)
            nc.tensor.matmul(out=ppool[:], lhsT=selB[:],
                             rhs=h2b[:, t * (HID + 1):(t + 1) * (HID + 1)],
                             start=(t == 0), stop=(t == T - 1))
        pool_sb = sb.tile([P, HID + 1], F32)
        nc.vector.tensor_copy(pool_sb[:], ppool[:])
        nc.gpsimd.dma_start(out=arin[:], in_=pool_sb[:])
        nc.gpsimd.collective_compute(
            "AllReduce", A.add, replica_groups=replica,
            ins=[arin.opt()], outs=[arout.opt()])
        sums = sb.tile([P, HID + 1], F32)
        nc.gpsimd.dma_start(out=sums[:], in_=arout[:])

        cnt = sb.tile([P, 1], F32)
        nc.vector.tensor_scalar(out=cnt[:], in0=sums[:, HID:HID + 1],
                                scalar1=1.0, op0=A.max, scalar2=None)
        rec = sb.tile([P, 1], F32)
        nc.vector.reciprocal(rec[:], cnt[:])
        y = sb.tile([P, HID], F32)
        nc.vector.tensor_tensor(out=y[:], in0=sums[:, :HID],
                                in1=rec[:].to_broadcast([P, HID]), op=A.mult)

        ur = sb.tile([P, HID], F32)
        rr_ = sb.tile([P, HID], F32)
        nc.vector.tensor_scalar(out=rr_[:], in0=y[:], scalar1=INV2PI,
                                scalar2=MAGIC, op0=A.mult, op1=A.add)
        nc.vector.tensor_scalar(out=rr_[:], in0=rr_[:], scalar1=MAGIC,
                                op0=A.subtract, scalar2=None)
        nc.vector.tensor_scalar(out=rr_[:], in0=rr_[:], scalar1=-TWOPI,
                                op0=A.mult, scalar2=None)
        nc.vector.tensor_tensor(out=ur[:], in0=y[:], in1=rr_[:], op=A.add)
        sy = sb.tile([P, HID], F32)
        chy = sb.tile([P, HID], F32)
        cy = sb.tile([P, HID], F32)
        nc.scalar.activation(sy[:], ur[:], ACT.Sin)
        nc.scalar.activation(chy[:], ur[:], ACT.Sin, scale=0.5)
        nc.vector.tensor_tensor(out=cy[:], in0=chy[:], in1=chy[:], op=A.mult)
        nc.vector.tensor_scalar(out=cy[:], in0=cy[:], scalar1=-2.0, scalar2=1.0,
                                op0=A.mult, op1=A.add)
        dotc = sb.tile([P, HID], F32)
        accc = sb.tile([P, 1], F32)
        nc.vector.tensor_tensor(out=dotc[:], in0=cy[:], in1=wro[:, 0:HID],
                                op=A.mult)
        nc.vector.tensor_reduce(out=accc[:], in_=dotc[:],
                                axis=mybir.AxisListType.X, op=A.add)
        dots = sb.tile([P, HID], F32)
        accs = sb.tile([P, 1], F32)
        nc.vector.tensor_tensor(out=dots[:], in0=sy[:], in1=wro[:, HID:2 * HID],
                                op=A.mult)
        nc.vector.tensor_reduce(out=accs[:], in_=dots[:],
                                axis=mybir.AxisListType.X, op=A.add)
        o_t = sb.tile([P, 1], F32)
        nc.vector.tensor_tensor(out=o_t[:], in0=accc[:], in1=accs[:], op=A.add)
        nc.vector.tensor_tensor(out=o_t[:], in0=o_t[:],
                                in1=wro[:, 2 * HID:2 * HID + 1], op=A.add)
        res = sb.tile([P, 1], F32)
        nc.scalar.activation(res[:], o_t[:], ACT.Sigmoid)
        nc.sync.dma_start(out=out_d[:], in_=res[:])

    nc.compile()
    return nc


def _make_runner(nc):
    import jax
    from jax.sharding import Mesh, PartitionSpec
    from jax.experimental.shard_map import shard_map
    import concourse.mybir as mybir
    from concourse import bass2jax

    bass2jax.install_neuronx_cc_hook()
    pname = nc.partition_id_tensor.name if nc.partition_id_tensor else None
    in_names, out_names, out_avals, zero_shapes = [], [], [], []
    for alloc in nc.m.functions[0].allocations:
        if not isinstance(alloc, mybir.MemoryLocationSet):
            continue
        name = alloc.memorylocations[0].name
        if alloc.kind == "ExternalInput":
            if name != pname:
                in_names.append(name)
        elif alloc.kind == "ExternalOutput":
            out_names.append(name)
            shape = tuple(alloc.tensor_shape)
            dtype = mybir.dt.np(alloc.dtype)
            out_avals.append(jax.core.ShapedArray(shape, dtype))
            zero_shapes.append((shape, dtype))
    n_params = len(in_names)
    all_names = in_names + out_names + ([pname] if pname else [])

    def _body(*args):
        operands = list(args)
        if pname is not None:
            operands.append(bass2jax.partition_id_tensor())
        outs = bass2jax._bass_exec_p.bind(
            *operands, out_avals=tuple(out_avals), in_names=tuple(all_names),
            out_names=tuple(out_names), lowering_input_output_aliases=(),
            sim_require_finite=True, sim_require_nnan=True, nc=nc)
        return tuple(outs)

    devices = jax.devices()[:N_CORES]
    mesh = Mesh(np.asarray(devices), ("core",))
    n_outs = len(out_names)
    in_specs = (PartitionSpec("core"),) * (n_params + n_outs)
    out_specs = (PartitionSpec("core"),) * n_outs
    donate = tuple(range(n_params, n_params + n_outs))
    sharded = jax.jit(
        shard_map(_body, mesh=mesh, in_specs=in_specs, out_specs=out_specs,
                  check_rep=False),
        donate_argnums=donate, keep_unused=True)

    def run(in_maps):
        concat_in = [
            np.concatenate([np.asarray(in_maps[c][nm]) for c in range(N_CORES)],
                           axis=0)
            for nm in in_names]
        concat_zeros = [np.zeros((N_CORES * s[0], *s[1:]), d)
                        for (s, d) in zero_shapes]
        out_arrs = sharded(*concat_in, *concat_zeros)
        return [
            {nm: np.asarray(out_arrs[i]).reshape(N_CORES, *out_avals[i].shape)[c]
             for i, nm in enumerate(out_names)}
            for c in range(N_CORES)]

    return run


def _warmup(run):
    import jax
    z = [{"xtb": np.zeros((INF, NPC), np.float16),
          "srcp": np.zeros((P, T * CH), np.uint16),
          "dlob": np.zeros((P, T * CH), np.float16),
          "batchb": np.zeros((P, T), np.float16),
          "win": np.zeros((4, 2 * INF, HID), np.float16),
          "wconv": np.zeros((N_CONV, 2, GRID * HID, HID), np.float16),
          "wro": np.zeros((P, 2 * HID + 1), np.float32)}
         for _ in range(N_CORES)]
    run(z)
    jax.effects_barrier()


def _get_runner():
    if _STATE.get("dead"):
        raise RuntimeError("device path disabled")
    if "run" not in _STATE:
        nc = _build_nc()
        run = _make_runner(nc)
        # warm-up: triggers neuronx-cc (disk-cached) + device program load.
        # A failure here usually means the device was left in a bad state by
        # a previous process; that first failed attempt resets it, so retry
        # once before giving up on the device path.
        try:
            _warmup(run)
        except Exception:
            _drain_tokens()
            try:
                _warmup(run)
            except Exception:
                _drain_tokens()
                _STATE["dead"] = True
                raise
        _STATE["run"] = run
    return _STATE["run"]


def _drain_tokens():
    try:
        import jax
        jax.effects_barrier()
    except Exception:
        pass


# ======================= host preprocessing =======================

def _preprocess(x, edge_index, batch, W_in, W_conv, W_out, b_out):
    src = np.ascontiguousarray(edge_index[0]).astype(np.int32)
    dst = np.ascontiguousarray(edge_index[1]).astype(np.int32)

    hi = (dst >> 7).astype(np.int16)
    perm = np.argsort(hi, kind="stable")
    src_s = src[perm]
    dst_s = dst[perm]
    n_buckets = N_CORES * T
    counts = np.bincount(hi, minlength=n_buckets).astype(np.int64)
    if counts.max() > CH * P:
        raise ValueError("bucket overflow")

    CAP = CH * P
    mask = np.arange(CAP)[None, :] < counts[:, None]
    srcpad = np.zeros((n_buckets, CAP), np.uint16)
    srcpad[mask] = src_s.astype(np.uint16)
    dlopad = np.full((n_buckets, CAP), 255, np.int16)
    dlopad[mask] = (dst_s & 127).astype(np.int16)
    srcpad = srcpad.reshape(n_buckets, CH, P)
    dlopad = dlopad.reshape(n_buckets, CH, P)

    batch_np = np.ascontiguousarray(batch).astype(np.int32)
    x32 = np.ascontiguousarray(x).astype(np.float32)
    n_nodes = x32.shape[0]

    KIN = 2 * INF
    KCV = GRID * HID
    W_in = np.asarray(W_in, np.float32)
    W_conv = np.asarray(W_conv, np.float32)
    W_out = np.asarray(W_out, np.float32)
    b_out = np.asarray(b_out, np.float32)
    win = np.zeros((4, KIN, HID), np.float16)
    for kk in range(GRID):
        blk, off = divmod(kk, 2)
        win[blk, off * INF:(off + 1) * INF, :] = W_in[0, :, :, kk].T
        win[2 + blk, off * INF:(off + 1) * INF, :] = W_in[1, :, :, kk].T
    wconv = np.zeros((N_CONV, 2, KCV, HID), np.float16)
    for l in range(N_CONV):
        for kk in range(GRID):
            wconv[l, 0, kk * HID:(kk + 1) * HID, :] = W_conv[l, 0, :, :, kk].T
            wconv[l, 1, kk * HID:(kk + 1) * HID, :] = W_conv[l, 1, :, :, kk].T
    wro = np.zeros((P, 2 * HID + 1), np.float32)
    wro[:, 0:HID] = W_out[0, 0, :, 0][None, :]
    wro[:, HID:2 * HID] = W_out[1, 0, :, 0][None, :]
    wro[:, 2 * HID] = b_out.ravel()[0]

    in_maps = []
    for c in range(N_CORES):
        lo = c * NPC
        xs = np.zeros((NPC, INF), np.float32)
        n_real = max(0, min(lo + NPC, n_nodes) - lo)
        if n_real > 0:
            xs[:n_real] = x32[lo:lo + n_real]
        xtb = np.ascontiguousarray(xs.T).astype(np.float16)

        bt = np.full(NPC, 255, np.int32)
        if n_real > 0:
            bt[:n_real] = batch_np[lo:lo + n_real]
        batchb = np.ascontiguousarray(bt.reshape(T, P).T).astype(np.float16)

        sp = srcpad[c * T:(c + 1) * T]
        dp = dlopad[c * T:(c + 1) * T]
        srcp = np.ascontiguousarray(sp.transpose(2, 0, 1).reshape(P, T * CH))
        dlob = np.ascontiguousarray(
            dp.transpose(2, 0, 1).reshape(P, T * CH)).astype(np.float16)

        in_maps.append({"xtb": xtb, "srcp": srcp, "dlob": dlob,
                        "batchb": batchb, "win": win, "wconv": wconv,
                        "wro": wro})
    return in_maps


# ======================= NumPy fallback =======================

def _cheb_trig(h):
    c1 = np.cos(h)
    s1 = np.sin(h)
    twoc = 2.0 * c1
    c2 = twoc * c1 - 1.0
    s2 = twoc * s1
    c3 = twoc * c2 - c1
    s3 = twoc * s2 - s1
    c4 = twoc * c3 - c2
    s4 = twoc * s3 - s2
    B = h.shape[0]
    co = np.concatenate([c1, c2, c3, c4], axis=1)
    si = np.concatenate([s1, s2, s3, s4], axis=1)
    return co, si


def _kan_fast(h, W, bias=None):
    # W: [2, out, in, 4] -> g-major column layout to match _cheb_trig concat
    out_dim = W.shape[1]
    W0 = np.ascontiguousarray(W[0].transpose(0, 2, 1).reshape(out_dim, -1))
    W1 = np.ascontiguousarray(W[1].transpose(0, 2, 1).reshape(out_dim, -1))
    co, si = _cheb_trig(h)
    y = co @ W0.T + si @ W1.T
    if bias is not None:
        y = y + bias
    return y.astype(np.float32)


def _fallback(x, edge_index, batch, W_in, W_conv, W_out, b_out):
    x = np.asarray(x, np.float32)
    src = np.asarray(edge_index[0]).astype(np.int32)
    dst = np.asarray(edge_index[1]).astype(np.int32)
    bat = np.asarray(batch).astype(np.int32)
    W_in = np.asarray(W_in, np.float32)
    W_conv = np.asarray(W_conv, np.float32)
    W_out = np.asarray(W_out, np.float32)
    b_out = np.asarray(b_out, np.float32)
    n_nodes = x.shape[0]
    n_edges = src.shape[0]

    order = np.argsort(dst, kind="stable")
    dst_s = dst[order]
    src_s = src[order]
    starts = np.searchsorted(dst_s, np.arange(n_nodes, dtype=np.int32))
    starts_c = np.minimum(starts, n_edges - 1).astype(np.int64)
    empty = np.bincount(dst, minlength=n_nodes) == 0

    h = _kan_fast(x, W_in)
    for l in range(N_CONV):
        msg = _kan_fast(h, W_conv[l])
        g = msg[src_s]
        m = np.add.reduceat(g, starts_c, axis=0)
        m[empty] = 0.0
        z = m + h
        h = np.where(z >= 0, z, NEG_SLOPE * z).astype(np.float32)

    # batch is sorted -> reduceat for pooling
    bstarts = np.searchsorted(bat, np.arange(N_GRAPHS, dtype=np.int32))
    bstarts_c = np.minimum(bstarts, n_nodes - 1).astype(np.int64)
    bcounts = np.bincount(bat, minlength=N_GRAPHS).astype(np.float32)
    sums = np.add.reduceat(h, bstarts_c, axis=0)
    sums[bcounts == 0] = 0.0
    y = sums / np.maximum(bcounts, 1.0)[:, None]

    # readout (grid=1)
    o = (np.cos(y) @ W_out[0, :, :, 0].T + np.sin(y) @ W_out[1, :, :, 0].T
         + b_out)
    return (1.0 / (1.0 + np.exp(-o.astype(np.float32)))).astype(np.float32)


# ======================= entry point =======================

def kernel(x, edge_index, batch, W_in, W_conv, W_out, b_out):
    try:
        import jax
        run = _get_runner()
        in_maps = _preprocess(x, edge_index, batch, W_in, W_conv, W_out, b_out)
        res = run(in_maps)
        out = np.asarray(res[0]["out"], np.float32)
        # Wait for full device-side completion (incl. kernel-tail barriers)
        # before returning: exiting the process mid-tail wedges the device.
        jax.effects_barrier()
        if out.shape != (N_GRAPHS, 1):
            raise ValueError("bad output shape")
        if not np.all(np.isfinite(out)) or out.min() < 0.0 or out.max() > 1.0:
            raise ValueError("output sanity check failed")
        if not np.array_equal(out, np.asarray(res[N_CORES - 1]["out"])):
            raise ValueError("cross-core mismatch")
        return out
    except Exception:
        _drain_tokens()
        return _fallback(x, edge_index, batch, W_in, W_conv, W_out, b_out)


# Build + compile + warm up at import so kernel() pays only per-input costs.
try:
    _get_runner()
except Exception:
    _STATE.pop("run", None)


# revision 7
# speedup vs baseline: 3.9984x; 1.1713x over previous
"""KA-GNN (Fourier-KAN message passing) on 8 Trainium2 NeuronCores via Bass.

Strategy (sharding_hint: partition nodes/edges across cores, replicate small
Fourier coefficients, collective only at readout):
 - nodes are split into 128-row tiles; each core owns T=49 consecutive tiles
   (dst buckets) -> node/edge partition across 8 cores.
 - per conv layer each core computes Fourier-KAN messages for its own nodes
   (Chebyshev recurrence on-device: cos/sin(k*h), k=1..4), writes a fp16 msg
   table slice to DRAM, AllGather -> full table.
 - scatter-add aggregation: edges bucketed by dst tile (host-side O(E) radix
   grouping), per 128-edge chunk one indirect-DMA row gather + one-hot
   selection matrix (is_equal vs iota) matmul accumulated in PSUM.
 - mean-pool partial sums per core -> AllReduce -> replicated KAN readout.

Import-time: trace + neuronx-cc compile (disk-cached) + warm-up execution.
kernel() call: host preprocessing + one sharded device invocation.
Any failure falls back to a pure-NumPy implementation.
"""

import numpy as np

P = 128
N_CORES = 8
T = 49                 # node tiles (dst buckets) per core
CH = 18                # max 128-edge chunks per bucket
INF = 64
HID = 32
GRID = 4
N_CONV = 2
NCH = 6                # trig free-dim chunking
N_NODES = 50000
N_GRAPHS = 128
NEG_SLOPE = 0.01
NPC = T * P
NPAD = NPC * N_CORES

MAGIC = 1.5 * 2.0**23
INV2PI = float(1.0 / (2.0 * np.pi))
TWOPI = float(2.0 * np.pi)

_STATE = {}


# ======================= device program =======================

def _build_nc():
    from contextlib import ExitStack
    import concourse.bass as bass
    import concourse.bacc as bacc
    import concourse.mybir as mybir
    import concourse.tile as tile
    from concourse.masks import make_identity

    F32 = mybir.dt.float32
    F16 = mybir.dt.float16
    I32 = mybir.dt.int32
    U16 = mybir.dt.uint16
    A = mybir.AluOpType
    ACT = mybir.ActivationFunctionType

    KIN = 2 * INF
    KCV = GRID * HID

    nc = bacc.Bacc("TRN2", target_bir_lowering=False, debug=False,
                   num_devices=N_CORES)

    xtb_d = nc.dram_tensor("xtb", [INF, NPC], F16, kind="ExternalInput").ap()
    srcp_d = nc.dram_tensor("srcp", [P, T * CH], U16, kind="ExternalInput").ap()
    dlob_d = nc.dram_tensor("dlob", [P, T * CH], F16, kind="ExternalInput").ap()
    batch_d = nc.dram_tensor("batchb", [P, T], F16, kind="ExternalInput").ap()
    win_d = nc.dram_tensor("win", [4, KIN, HID], F16, kind="ExternalInput").ap()
    wconv_d = nc.dram_tensor("wconv", [N_CONV, 2, KCV, HID], F16,
                             kind="ExternalInput").ap()
    wro_d = nc.dram_tensor("wro", [P, 2 * HID + 1], F32, kind="ExternalInput").ap()
    out_d = nc.dram_tensor("out", [P, 1], F32, kind="ExternalOutput").ap()

    replica = [list(range(N_CORES))]
    CHW = (T + NCH - 1) // NCH

    with tile.TileContext(nc) as tc, ExitStack() as ctx:
        sb = ctx.enter_context(tc.tile_pool(name="sb", bufs=1))
        tg = ctx.enter_context(tc.tile_pool(name="tg", bufs=1))
        sbg = ctx.enter_context(tc.tile_pool(name="sbg", bufs=4))
        sel_p = ctx.enter_context(tc.tile_pool(name="selp", bufs=2))
        pst = ctx.enter_context(tc.tile_pool(name="pst", bufs=2, space="PSUM"))
        psa = ctx.enter_context(tc.tile_pool(name="psa", bufs=2, space="PSUM"))
        dr = ctx.enter_context(tc.tile_pool(name="dr", bufs=1, space="DRAM"))

        # ---------------- setup ----------------
        xtb = sb.tile([INF, NPC], F16)
        nc.sync.dma_start(out=xtb[:], in_=xtb_d[:])
        srcp_u = sb.tile([P, T * CH], U16)
        nc.sync.dma_start(out=srcp_u[:], in_=srcp_d[:])
        srci = sb.tile([P, T * CH], I32)
        nc.vector.tensor_copy(srci[:], srcp_u[:])
        dlo = sb.tile([P, T * CH], F16)
        nc.sync.dma_start(out=dlo[:], in_=dlob_d[:])
        batchb = sb.tile([P, T], F16)
        nc.sync.dma_start(out=batchb[:], in_=batch_d[:])

        win_b = sb.tile([P, 4 * HID], F16)
        for b in range(4):
            nc.sync.dma_start(out=win_b[:KIN, b * HID:(b + 1) * HID], in_=win_d[b])
        wconv_b = sb.tile([P, N_CONV * 2 * HID], F16)
        for l in range(N_CONV):
            for s in range(2):
                c0 = (l * 2 + s) * HID
                nc.sync.dma_start(out=wconv_b[:KCV, c0:c0 + HID], in_=wconv_d[l, s])
        wro = sb.tile([P, 2 * HID + 1], F32)
        nc.sync.dma_start(out=wro[:], in_=wro_d[:])

        iota_i = sb.tile([P, P], I32)
        nc.gpsimd.iota(iota_i[:], pattern=[[1, P]], base=0, channel_multiplier=0)
        iota_h = sb.tile([P, P], F16)
        nc.vector.tensor_copy(iota_h[:], iota_i[:])
        ident = sb.tile([P, P], F32)
        make_identity(nc, ident[:])

        locl = [dr.tile([NPC, HID], F16, name=f"locl{i}") for i in range(N_CONV)]
        tbl = [dr.tile([NPAD, HID], F16, name=f"tbl{i}") for i in range(N_CONV)]
        arin = dr.tile([P, HID + 1], F32)
        arout = dr.tile([P, HID + 1], F32)

        h_buf_a = sb.tile([P, T * HID], F32, tag="hbuf_a")
        h_buf_b = sb.tile([P, T * HID], F32, tag="hbuf_b")
        TcA = sb.tile([P, NPC], F16, tag="TcA")
        TcB = sb.tile([P, NPC], F16, tag="TcB")
        TsA = sb.tile([P, NPC], F16, tag="TsA")
        TsB = sb.tile([P, NPC], F16, tag="TsB")

        def chebyshev(src, rows, cols, col0, dst_of):
            u = tg.tile([rows, CHW * P], F32, tag="ch_u")
            r = tg.tile([rows, CHW * P], F32, tag="ch_r")
            nc.vector.tensor_scalar(out=r[:rows, :cols], in0=src, scalar1=INV2PI,
                                    scalar2=MAGIC, op0=A.mult, op1=A.add)
            nc.vector.tensor_scalar(out=r[:rows, :cols], in0=r[:rows, :cols],
                                    scalar1=MAGIC, op0=A.subtract, scalar2=None)
            nc.vector.tensor_scalar(out=r[:rows, :cols], in0=r[:rows, :cols],
                                    scalar1=-TWOPI, op0=A.mult, scalar2=None)
            nc.vector.tensor_tensor(out=u[:rows, :cols], in0=src,
                                    in1=r[:rows, :cols], op=A.add)
            s1 = tg.tile([rows, CHW * P], F32, tag="ch_s1")
            chh = tg.tile([rows, CHW * P], F32, tag="ch_ch")
            c1 = tg.tile([rows, CHW * P], F32, tag="ch_c1")
            nc.scalar.activation(s1[:rows, :cols], u[:rows, :cols], ACT.Sin)
            nc.scalar.activation(chh[:rows, :cols], u[:rows, :cols], ACT.Sin,
                                 scale=0.5)
            nc.vector.tensor_tensor(out=c1[:rows, :cols], in0=chh[:rows, :cols],
                                    in1=chh[:rows, :cols], op=A.mult)
            nc.vector.tensor_scalar(out=c1[:rows, :cols], in0=c1[:rows, :cols],
                                    scalar1=-2.0, scalar2=1.0, op0=A.mult,
                                    op1=A.add)
            twoc = tg.tile([rows, CHW * P], F32, tag="ch_twoc")
            nc.vector.tensor_scalar(out=twoc[:rows, :cols], in0=c1[:rows, :cols],
                                    scalar1=2.0, op0=A.mult, scalar2=None)
            cs = {1: (c1, s1)}
            prev_c, prev_s = c1, s1
            pprev_c, pprev_s = None, None
            for k in range(2, GRID + 1):
                ck = tg.tile([rows, CHW * P], F32, tag=f"ch_c{k}")
                sk = tg.tile([rows, CHW * P], F32, tag=f"ch_s{k}")
                nc.vector.tensor_tensor(out=ck[:rows, :cols],
                                        in0=twoc[:rows, :cols],
                                        in1=prev_c[:rows, :cols], op=A.mult)
                nc.vector.tensor_tensor(out=sk[:rows, :cols],
                                        in0=twoc[:rows, :cols],
                                        in1=prev_s[:rows, :cols], op=A.mult)
                if k == 2:
                    nc.vector.tensor_scalar(out=ck[:rows, :cols],
                                            in0=ck[:rows, :cols], scalar1=1.0,
                                            op0=A.subtract, scalar2=None)
                else:
                    nc.vector.tensor_tensor(out=ck[:rows, :cols],
                                            in0=ck[:rows, :cols],
                                            in1=pprev_c[:rows, :cols],
                                            op=A.subtract)
                    nc.vector.tensor_tensor(out=sk[:rows, :cols],
                                            in0=sk[:rows, :cols],
                                            in1=pprev_s[:rows, :cols],
                                            op=A.subtract)
                cs[k] = (ck, sk)
                pprev_c, pprev_s = prev_c, prev_s
                prev_c, prev_s = ck, sk
            for k in range(1, GRID + 1):
                (ctile, r0), (stile, r1) = dst_of(k)
                ckf, skf = cs[k]
                nc.vector.tensor_copy(ctile[r0:r0 + rows, col0:col0 + cols],
                                      ckf[:rows, :cols])
                nc.vector.tensor_copy(stile[r1:r1 + rows, col0:col0 + cols],
                                      skf[:rows, :cols])

        # ---------- input layer ----------
        def in_dst(k):
            if k <= 2:
                return (TcA, (k - 1) * INF), (TsA, (k - 1) * INF)
            return (TcB, (k - 3) * INF), (TsB, (k - 3) * INF)

        for c0 in range(0, T, CHW):
            c1_ = min(c0 + CHW, T)
            cols = (c1_ - c0) * P
            chebyshev(xtb[:INF, c0 * P:c0 * P + cols], INF, cols, c0 * P, in_dst)

        h_cur, h_nxt = h_buf_a, h_buf_b
        for t in range(T):
            ph = psa.tile([P, HID], F32, tag="ps_acc")
            blocks = [(TcA, 0), (TcB, 1), (TsA, 2), (TsB, 3)]
            for i, (tt, b) in enumerate(blocks):
                nc.tensor.matmul(out=ph[:], lhsT=tt[:KIN, t * P:(t + 1) * P],
                                 rhs=win_b[:KIN, b * HID:(b + 1) * HID],
                                 start=(i == 0), stop=(i == 3))
            nc.vector.tensor_copy(h_cur[:, t * HID:(t + 1) * HID], ph[:])

        # ---------- conv layers ----------
        def conv_dst(k):
            return (TcA, (k - 1) * HID), (TcB, (k - 1) * HID)

        for l in range(N_CONV):
            for c0 in range(0, T, CHW):
                c1_ = min(c0 + CHW, T)
                cols = (c1_ - c0) * P
                h_T = tg.tile([HID, CHW * P], F32, tag="h_T")
                for t in range(c0, c1_):
                    pt = pst.tile([HID, P], F32, tag="ps_tr")
                    nc.tensor.transpose(out=pt[:],
                                        in_=h_cur[:, t * HID:(t + 1) * HID],
                                        identity=ident[:])
                    nc.vector.tensor_copy(
                        h_T[:, (t - c0) * P:(t - c0 + 1) * P], pt[:])
                chebyshev(h_T[:HID, :cols], HID, cols, c0 * P, conv_dst)

            msg_sb = sb.tile([P, T * HID], F16, tag="msg_sb")
            for t in range(T):
                pm = psa.tile([P, HID], F32, tag="ps_acc")
                nc.tensor.matmul(out=pm[:], lhsT=TcA[:KCV, t * P:(t + 1) * P],
                                 rhs=wconv_b[:KCV, (l * 2) * HID:(l * 2 + 1) * HID],
                                 start=True, stop=False)
                nc.tensor.matmul(out=pm[:], lhsT=TcB[:KCV, t * P:(t + 1) * P],
                                 rhs=wconv_b[:KCV, (l * 2 + 1) * HID:(l * 2 + 2) * HID],
                                 start=False, stop=True)
                nc.vector.tensor_copy(msg_sb[:, t * HID:(t + 1) * HID], pm[:])
            nc.gpsimd.dma_start(
                out=locl[l][:].rearrange("(t p) f -> p t f", p=P),
                in_=msg_sb[:].rearrange("p (t f) -> p t f", t=T))
            nc.gpsimd.collective_compute(
                "AllGather", A.bypass, replica_groups=replica,
                ins=[locl[l].opt()], outs=[tbl[l].opt()])

            for b in range(T):
                sel = sel_p.tile([P, CH * P], F16, tag="sel")
                nc.vector.tensor_tensor(
                    out=sel[:].rearrange("p (c q) -> p c q", c=CH),
                    in0=dlo[:, b * CH:(b + 1) * CH]
                        .rearrange("p (c o) -> p c o", o=1)
                        .to_broadcast([P, CH, P]),
                    in1=iota_h[:].rearrange("p (o q) -> p o q", o=1)
                        .to_broadcast([P, CH, P]),
                    op=A.is_equal)
                pagg = psa.tile([P, HID], F32, tag="ps_agg")
                for j in range(CH):
                    g_t = sbg.tile([P, HID], F16, tag="gath")
                    nc.gpsimd.indirect_dma_start(
                        out=g_t[:], out_offset=None, in_=tbl[l][:],
                        in_offset=bass.IndirectOffsetOnAxis(
                            ap=srci[:, b * CH + j:b * CH + j + 1], axis=0))
                    nc.tensor.matmul(out=pagg[:],
                                     lhsT=sel[:, j * P:(j + 1) * P],
                                     rhs=g_t[:], start=(j == 0),
                                     stop=(j == CH - 1))
                z = sbg.tile([P, HID], F32, tag="zt")
                nc.vector.tensor_tensor(out=z[:], in0=pagg[:],
                                        in1=h_cur[:, b * HID:(b + 1) * HID],
                                        op=A.add)
                zs = sbg.tile([P, HID], F32, tag="zs")
                nc.vector.tensor_scalar(out=zs[:], in0=z[:], scalar1=NEG_SLOPE,
                                        op0=A.mult, scalar2=None)
                nc.vector.tensor_tensor(out=h_nxt[:, b * HID:(b + 1) * HID],
                                        in0=z[:], in1=zs[:], op=A.max)
            h_cur, h_nxt = h_nxt, h_cur

        # ---------- pooling ----------
        h2b = sb.tile([P, T * (HID + 1)], F16, tag="h2b")
        for t in range(T):
            nc.vector.tensor_copy(
                h2b[:, t * (HID + 1):t * (HID + 1) + HID],
                h_cur[:, t * HID:(t + 1) * HID])
            nc.gpsimd.memset(h2b[:, t * (HID + 1) + HID:(t + 1) * (HID + 1)], 1.0)
        ppool = psa.tile([P, HID + 1], F32, tag="ps_acc")
        for t in range(T):
            selB = sbg.tile([P, P], F16, tag="selB")
            nc.vector.tensor_tensor(
                out=selB[:],
                in0=batchb[:, t:t + 1].to_broadcast([P, P]),
                in1=iota_h[:], op=A.is_equal)
            nc.tensor.matmul(out=ppool[:], lhsT=selB[:],
                             rhs=h2b[:, t * (HID + 1):(t + 1) * (HID + 1)],
                             start=(t == 0), stop=(t == T - 1))
        pool_sb = sb.tile([P, HID + 1], F32)
        nc.vector.tensor_copy(pool_sb[:], ppool[:])
        nc.gpsimd.dma_start(out=arin[:], in_=pool_sb[:])
        nc.gpsimd.collective_compute(
            "AllReduce", A.add, replica_groups=replica,
            ins=[arin.opt()], outs=[arout.opt()])
        sums = sb.tile([P, HID + 1], F32)
        nc.gpsimd.dma_start(out=sums[:], in_=arout[:])

        cnt = sb.tile([P, 1], F32)
        nc.vector.tensor_scalar(out=cnt[:], in0=sums[:, HID:HID + 1],
                                scalar1=1.0, op0=A.max, scalar2=None)
        rec = sb.tile([P, 1], F32)
        nc.vector.reciprocal(rec[:], cnt[:])
        y = sb.tile([P, HID], F32)
        nc.vector.tensor_tensor(out=y[:], in0=sums[:, :HID],
                                in1=rec[:].to_broadcast([P, HID]), op=A.mult)

        ur = sb.tile([P, HID], F32)
        rr_ = sb.tile([P, HID], F32)
        nc.vector.tensor_scalar(out=rr_[:], in0=y[:], scalar1=INV2PI,
                                scalar2=MAGIC, op0=A.mult, op1=A.add)
        nc.vector.tensor_scalar(out=rr_[:], in0=rr_[:], scalar1=MAGIC,
                                op0=A.subtract, scalar2=None)
        nc.vector.tensor_scalar(out=rr_[:], in0=rr_[:], scalar1=-TWOPI,
                                op0=A.mult, scalar2=None)
        nc.vector.tensor_tensor(out=ur[:], in0=y[:], in1=rr_[:], op=A.add)
        sy = sb.tile([P, HID], F32)
        chy = sb.tile([P, HID], F32)
        cy = sb.tile([P, HID], F32)
        nc.scalar.activation(sy[:], ur[:], ACT.Sin)
        nc.scalar.activation(chy[:], ur[:], ACT.Sin, scale=0.5)
        nc.vector.tensor_tensor(out=cy[:], in0=chy[:], in1=chy[:], op=A.mult)
        nc.vector.tensor_scalar(out=cy[:], in0=cy[:], scalar1=-2.0, scalar2=1.0,
                                op0=A.mult, op1=A.add)
        dotc = sb.tile([P, HID], F32)
        accc = sb.tile([P, 1], F32)
        nc.vector.tensor_tensor(out=dotc[:], in0=cy[:], in1=wro[:, 0:HID],
                                op=A.mult)
        nc.vector.tensor_reduce(out=accc[:], in_=dotc[:],
                                axis=mybir.AxisListType.X, op=A.add)
        dots = sb.tile([P, HID], F32)
        accs = sb.tile([P, 1], F32)
        nc.vector.tensor_tensor(out=dots[:], in0=sy[:], in1=wro[:, HID:2 * HID],
                                op=A.mult)
        nc.vector.tensor_reduce(out=accs[:], in_=dots[:],
                                axis=mybir.AxisListType.X, op=A.add)
        o_t = sb.tile([P, 1], F32)
        nc.vector.tensor_tensor(out=o_t[:], in0=accc[:], in1=accs[:], op=A.add)
        nc.vector.tensor_tensor(out=o_t[:], in0=o_t[:],
                                in1=wro[:, 2 * HID:2 * HID + 1], op=A.add)
        res = sb.tile([P, 1], F32)
        nc.scalar.activation(res[:], o_t[:], ACT.Sigmoid)
        nc.sync.dma_start(out=out_d[:], in_=res[:])

    nc.compile()
    return nc


def _make_runner(nc):
    import jax
    from jax.sharding import Mesh, PartitionSpec
    from jax.experimental.shard_map import shard_map
    import concourse.mybir as mybir
    from concourse import bass2jax

    bass2jax.install_neuronx_cc_hook()
    pname = nc.partition_id_tensor.name if nc.partition_id_tensor else None
    in_names, out_names, out_avals, zero_shapes = [], [], [], []
    for alloc in nc.m.functions[0].allocations:
        if not isinstance(alloc, mybir.MemoryLocationSet):
            continue
        name = alloc.memorylocations[0].name
        if alloc.kind == "ExternalInput":
            if name != pname:
                in_names.append(name)
        elif alloc.kind == "ExternalOutput":
            out_names.append(name)
            shape = tuple(alloc.tensor_shape)
            dtype = mybir.dt.np(alloc.dtype)
            out_avals.append(jax.core.ShapedArray(shape, dtype))
            zero_shapes.append((shape, dtype))
    n_params = len(in_names)
    all_names = in_names + out_names + ([pname] if pname else [])

    def _body(*args):
        operands = list(args)
        if pname is not None:
            operands.append(bass2jax.partition_id_tensor())
        outs = bass2jax._bass_exec_p.bind(
            *operands, out_avals=tuple(out_avals), in_names=tuple(all_names),
            out_names=tuple(out_names), lowering_input_output_aliases=(),
            sim_require_finite=True, sim_require_nnan=True, nc=nc)
        return tuple(outs)

    devices = jax.devices()[:N_CORES]
    mesh = Mesh(np.asarray(devices), ("core",))
    n_outs = len(out_names)
    in_specs = (PartitionSpec("core"),) * (n_params + n_outs)
    out_specs = (PartitionSpec("core"),) * n_outs
    donate = tuple(range(n_params, n_params + n_outs))
    sharded = jax.jit(
        shard_map(_body, mesh=mesh, in_specs=in_specs, out_specs=out_specs,
                  check_rep=False),
        donate_argnums=donate, keep_unused=True)

    core_sharding = jax.sharding.NamedSharding(mesh, PartitionSpec("core"))

    def run(in_maps):
        concat_in = [
            in_maps[0][nm] if isinstance(in_maps[0][nm], jax.Array)
            else np.concatenate(
                [np.asarray(in_maps[c][nm]) for c in range(N_CORES)], axis=0)
            for nm in in_names]
        concat_zeros = [np.zeros((N_CORES * s[0], *s[1:]), d)
                        for (s, d) in zero_shapes]
        out_arrs = sharded(*concat_in, *concat_zeros)
        return [
            {nm: np.asarray(out_arrs[i]).reshape(N_CORES, *out_avals[i].shape)[c]
             for i, nm in enumerate(out_names)}
            for c in range(N_CORES)]

    run.core_sharding = core_sharding
    return run


def _warmup(run):
    import jax
    z = [{"xtb": None,
          "srcp": np.zeros((P, T * CH), np.uint16),
          "dlob": np.zeros((P, T * CH), np.float16),
          "batchb": np.zeros((P, T), np.float16),
          "win": np.zeros((4, 2 * INF, HID), np.float16),
          "wconv": np.zeros((N_CONV, 2, GRID * HID, HID), np.float16),
          "wro": np.zeros((P, 2 * HID + 1), np.float32)}
         for _ in range(N_CORES)]
    xz = jax.device_put(np.zeros((N_CORES * INF, NPC), np.float16),
                        run.core_sharding)
    for m in z:
        m["xtb"] = xz
    run(z)
    jax.effects_barrier()


def _get_runner():
    if _STATE.get("dead"):
        raise RuntimeError("device path disabled")
    if "run" not in _STATE:
        nc = _build_nc()
        run = _make_runner(nc)
        # warm-up: triggers neuronx-cc (disk-cached) + device program load.
        # A failure here usually means the device was left in a bad state by
        # a previous process; that first failed attempt resets it, so retry
        # once before giving up on the device path.
        try:
            _warmup(run)
        except Exception:
            _drain_tokens()
            try:
                _warmup(run)
            except Exception:
                _drain_tokens()
                _STATE["dead"] = True
                raise
        _STATE["run"] = run
    return _STATE["run"]


def _drain_tokens():
    try:
        import jax
        jax.effects_barrier()
    except Exception:
        pass


# ======================= host preprocessing =======================

def _prep_x(x):
    """Concatenated, transposed, fp16-cast x for all cores: [N_CORES*INF, NPC]."""
    x32 = np.ascontiguousarray(np.asarray(x), np.float32)
    n_nodes = x32.shape[0]
    xall = np.zeros((N_CORES, NPC, INF), np.float32)
    flat = xall.reshape(N_CORES * NPC, INF)
    flat[:n_nodes] = x32
    return np.ascontiguousarray(
        xall.transpose(0, 2, 1)).astype(np.float16).reshape(N_CORES * INF, NPC)


def _preprocess(x, edge_index, batch, W_in, W_conv, W_out, b_out):
    src = np.ascontiguousarray(edge_index[0]).astype(np.int32)
    dst = np.ascontiguousarray(edge_index[1]).astype(np.int32)

    hi = (dst >> 7).astype(np.int16)
    perm = np.argsort(hi, kind="stable")
    src_s = src[perm]
    dst_s = dst[perm]
    n_buckets = N_CORES * T
    counts = np.bincount(hi, minlength=n_buckets).astype(np.int64)
    if counts.max() > CH * P:
        raise ValueError("bucket overflow")

    CAP = CH * P
    mask = np.arange(CAP)[None, :] < counts[:, None]
    srcpad = np.zeros((n_buckets, CAP), np.uint16)
    srcpad[mask] = src_s.astype(np.uint16)
    dlopad = np.full((n_buckets, CAP), 255, np.int16)
    dlopad[mask] = (dst_s & 127).astype(np.int16)
    srcpad = srcpad.reshape(n_buckets, CH, P)
    dlopad = dlopad.reshape(n_buckets, CH, P)

    batch_np = np.ascontiguousarray(batch).astype(np.int32)
    n_nodes = np.asarray(x).shape[0]

    KIN = 2 * INF
    KCV = GRID * HID
    W_in = np.asarray(W_in, np.float32)
    W_conv = np.asarray(W_conv, np.float32)
    W_out = np.asarray(W_out, np.float32)
    b_out = np.asarray(b_out, np.float32)
    win = np.zeros((4, KIN, HID), np.float16)
    for kk in range(GRID):
        blk, off = divmod(kk, 2)
        win[blk, off * INF:(off + 1) * INF, :] = W_in[0, :, :, kk].T
        win[2 + blk, off * INF:(off + 1) * INF, :] = W_in[1, :, :, kk].T
    wconv = np.zeros((N_CONV, 2, KCV, HID), np.float16)
    for l in range(N_CONV):
        for kk in range(GRID):
            wconv[l, 0, kk * HID:(kk + 1) * HID, :] = W_conv[l, 0, :, :, kk].T
            wconv[l, 1, kk * HID:(kk + 1) * HID, :] = W_conv[l, 1, :, :, kk].T
    wro = np.zeros((P, 2 * HID + 1), np.float32)
    wro[:, 0:HID] = W_out[0, 0, :, 0][None, :]
    wro[:, HID:2 * HID] = W_out[1, 0, :, 0][None, :]
    wro[:, 2 * HID] = b_out.ravel()[0]

    in_maps = []
    for c in range(N_CORES):
        lo = c * NPC
        n_real = max(0, min(lo + NPC, n_nodes) - lo)
        bt = np.full(NPC, 255, np.int32)
        if n_real > 0:
            bt[:n_real] = batch_np[lo:lo + n_real]
        batchb = np.ascontiguousarray(bt.reshape(T, P).T).astype(np.float16)

        sp = srcpad[c * T:(c + 1) * T]
        dp = dlopad[c * T:(c + 1) * T]
        srcp = np.ascontiguousarray(sp.transpose(2, 0, 1).reshape(P, T * CH))
        dlob = np.ascontiguousarray(
            dp.transpose(2, 0, 1).reshape(P, T * CH)).astype(np.float16)

        in_maps.append({"srcp": srcp, "dlob": dlob,
                        "batchb": batchb, "win": win, "wconv": wconv,
                        "wro": wro})
    return in_maps


# ======================= NumPy fallback =======================

def _cheb_trig(h):
    c1 = np.cos(h)
    s1 = np.sin(h)
    twoc = 2.0 * c1
    c2 = twoc * c1 - 1.0
    s2 = twoc * s1
    c3 = twoc * c2 - c1
    s3 = twoc * s2 - s1
    c4 = twoc * c3 - c2
    s4 = twoc * s3 - s2
    B = h.shape[0]
    co = np.concatenate([c1, c2, c3, c4], axis=1)
    si = np.concatenate([s1, s2, s3, s4], axis=1)
    return co, si


def _kan_fast(h, W, bias=None):
    # W: [2, out, in, 4] -> g-major column layout to match _cheb_trig concat
    out_dim = W.shape[1]
    W0 = np.ascontiguousarray(W[0].transpose(0, 2, 1).reshape(out_dim, -1))
    W1 = np.ascontiguousarray(W[1].transpose(0, 2, 1).reshape(out_dim, -1))
    co, si = _cheb_trig(h)
    y = co @ W0.T + si @ W1.T
    if bias is not None:
        y = y + bias
    return y.astype(np.float32)


def _fallback(x, edge_index, batch, W_in, W_conv, W_out, b_out):
    x = np.asarray(x, np.float32)
    src = np.asarray(edge_index[0]).astype(np.int32)
    dst = np.asarray(edge_index[1]).astype(np.int32)
    bat = np.asarray(batch).astype(np.int32)
    W_in = np.asarray(W_in, np.float32)
    W_conv = np.asarray(W_conv, np.float32)
    W_out = np.asarray(W_out, np.float32)
    b_out = np.asarray(b_out, np.float32)
    n_nodes = x.shape[0]
    n_edges = src.shape[0]

    order = np.argsort(dst, kind="stable")
    dst_s = dst[order]
    src_s = src[order]
    starts = np.searchsorted(dst_s, np.arange(n_nodes, dtype=np.int32))
    starts_c = np.minimum(starts, n_edges - 1).astype(np.int64)
    empty = np.bincount(dst, minlength=n_nodes) == 0

    h = _kan_fast(x, W_in)
    for l in range(N_CONV):
        msg = _kan_fast(h, W_conv[l])
        g = msg[src_s]
        m = np.add.reduceat(g, starts_c, axis=0)
        m[empty] = 0.0
        z = m + h
        h = np.where(z >= 0, z, NEG_SLOPE * z).astype(np.float32)

    # batch is sorted -> reduceat for pooling
    bstarts = np.searchsorted(bat, np.arange(N_GRAPHS, dtype=np.int32))
    bstarts_c = np.minimum(bstarts, n_nodes - 1).astype(np.int64)
    bcounts = np.bincount(bat, minlength=N_GRAPHS).astype(np.float32)
    sums = np.add.reduceat(h, bstarts_c, axis=0)
    sums[bcounts == 0] = 0.0
    y = sums / np.maximum(bcounts, 1.0)[:, None]

    # readout (grid=1)
    o = (np.cos(y) @ W_out[0, :, :, 0].T + np.sin(y) @ W_out[1, :, :, 0].T
         + b_out)
    return (1.0 / (1.0 + np.exp(-o.astype(np.float32)))).astype(np.float32)


# ======================= entry point =======================

def kernel(x, edge_index, batch, W_in, W_conv, W_out, b_out):
    try:
        import jax
        run = _get_runner()
        xd = jax.device_put(_prep_x(x), run.core_sharding)  # async upload
        in_maps = _preprocess(x, edge_index, batch, W_in, W_conv, W_out, b_out)
        for m in in_maps:
            m["xtb"] = xd
        res = run(in_maps)
        out = np.asarray(res[0]["out"], np.float32)
        # Wait for full device-side completion (incl. kernel-tail barriers)
        # before returning: exiting the process mid-tail wedges the device.
        jax.effects_barrier()
        if out.shape != (N_GRAPHS, 1):
            raise ValueError("bad output shape")
        if not np.all(np.isfinite(out)) or out.min() < 0.0 or out.max() > 1.0:
            raise ValueError("output sanity check failed")
        if not np.array_equal(out, np.asarray(res[N_CORES - 1]["out"])):
            raise ValueError("cross-core mismatch")
        return out
    except Exception:
        _drain_tokens()
        return _fallback(x, edge_index, batch, W_in, W_conv, W_out, b_out)


# Build + compile + warm up at import so kernel() pays only per-input costs.
try:
    _get_runner()
except Exception:
    _STATE.pop("run", None)
